# revision 12
# baseline (speedup 1.0000x reference)
"""Trainium2 Bass kernel for nn_ChildHAggregation (gnn_message_passing).

Per-sample math (B=32768, HALF=512, DIM=1024):
  x = [hl, hr]; 2-token attention with HyperLinear q/k; layernorm;
  out = hidden(x_norm, xh) + leaf(xw, xh)   (both HyperLinear)

Design (v2): pure data-parallel, batch-major [128 x feat] tiles.
Key algebra (hs/hd basis):
  hs = hl + hr, hd = hl - hr
  q_l/q_r projections: AS = hs@qU, AD = hd@qU (A_l = (AS+AD)/2 ...)
  k_l - k_r = (hd@kU) * tu   (biases/additive hyper cancel)
  d0/d1 (score diffs) -> p00, p11 via two-term softsign sigmoid:
    sigma(z) ~= 0.5 + a1*z/sqrt(z^2+c1) + (0.5-a1)*z/sqrt(z^2+c2)
  attention rows: u_c = hs + alpha*hd (alpha=p00), v_c = hs + beta*hd
  (beta=-p11), so with WS=top+bot, WT=top, WB=bot of alpha-folded hU:
    x@hU_a = hs@WS + alpha*(hd@WT) + beta*(hd@WB)   <- attention-free mms
  layernorm stats from accumulated row sums of hs/hd (ddof=1).
  Output-side hyper biases folded into the matmul accumulation via an
  identity-stationary matmul of the broadcast bias tile (no K=1 mms).
All matmul operands fp16 (same PE speed as bf16, 8x finer mantissa).
"""

import os
from contextlib import ExitStack

import numpy as np

import concourse.bacc as bacc
import concourse.bass as bass
import concourse.mybir as mybir
import concourse.tile as tile
from concourse.bass_utils import run_bass_kernel_spmd
from concourse.masks import make_identity

N_CORES = 8
B_FULL = 32768
HALF = 512
DIM = 1024
P = 128
IS = 1.0 / float(np.sqrt(np.float32(HALF)))

# two-term softsign sigmoid constants (max |err| 1.9e-3 over |z|<=14)
SIG_A1 = 2.057838
SIG_C1 = 8.347378
SIG_A2 = 0.5 - SIG_A1
SIG_C2 = 11.527823
SIG_K1 = SIG_A1 * IS / float(np.sqrt(SIG_C1))
SIG_K2 = SIG_A2 * IS / float(np.sqrt(SIG_C2))

f32 = mybir.dt.float32
fp16 = mybir.dt.float16

AX = mybir.AxisListType
ALU = mybir.AluOpType
ACTF = mybir.ActivationFunctionType

W512 = ["qU", "kU", "qWu", "qWb", "kWu", "kWb", "hWu", "hWb", "lWu", "lWb"]


def build_nc(b_loc):
    """Per-core Bass program for a local batch of b_loc rows."""
    n_tiles = b_loc // P
    assert n_tiles * P == b_loc

    nc = bacc.Bacc("TRN2", target_bir_lowering=False, debug=False,
                   num_devices=1)

    # ---- DRAM I/O (names match setup_inputs) ----
    d = {}
    d["hl"] = nc.dram_tensor("hl", [b_loc, HALF], f32, kind="ExternalInput").ap()
    d["hr"] = nc.dram_tensor("hr", [b_loc, HALF], f32, kind="ExternalInput").ap()
    d["xw"] = nc.dram_tensor("xw", [b_loc, DIM], f32, kind="ExternalInput").ap()
    d["xh"] = nc.dram_tensor("xh", [b_loc, HALF], f32, kind="ExternalInput").ap()
    for w in W512:
        d[w + "_w"] = nc.dram_tensor(w + "_w", [HALF, HALF], f32,
                                     kind="ExternalInput").ap()
        d[w + "_b"] = nc.dram_tensor(w + "_b", [HALF], f32,
                                     kind="ExternalInput").ap()
    for w in ["hU", "lU"]:
        d[w + "_w"] = nc.dram_tensor(w + "_w", [DIM, HALF], f32,
                                     kind="ExternalInput").ap()
        d[w + "_b"] = nc.dram_tensor(w + "_b", [HALF], f32,
                                     kind="ExternalInput").ap()
    d["alpha"] = nc.dram_tensor("alpha", [DIM], f32, kind="ExternalInput").ap()
    d["beta"] = nc.dram_tensor("beta", [DIM], f32, kind="ExternalInput").ap()
    out_d = nc.dram_tensor("out", [b_loc, HALF], f32, kind="ExternalOutput").ap()

    with tile.TileContext(nc) as tc, ExitStack() as ctx:
        # ================= persistent pools =================
        wts = ctx.enter_context(tc.tile_pool(name="wts", bufs=1))
        biasp = ctx.enter_context(tc.tile_pool(name="biasp", bufs=1))

        # fp16 weight tiles
        wsb = {}
        for w in ["qU", "kU", "qWu", "qWb", "kWu", "hWu", "lWu", "WC",
                  "WS", "WT", "WB"]:
            wsb[w] = wts.tile([P, 4, HALF], fp16, name=f"w_{w}")
        wsb["lU"] = wts.tile([P, 8, HALF], fp16, name="w_lU")
        alpha_sb = wts.tile([P, 8], f32)
        nc.sync.dma_start(alpha_sb, d["alpha"].rearrange("(c p) -> p c", p=P))
        beta_sb = wts.tile([P, 8], f32)
        nc.sync.dma_start(beta_sb, d["beta"].rearrange("(c p) -> p c", p=P))
        ident = wts.tile([P, P], fp16)
        make_identity(nc, ident)

        # persistent broadcast [P, 512] bias tiles
        bc = {}
        for nm in ["qWu_b", "kWu_b", "qb", "cs"]:
            bc[nm] = biasp.tile([P, HALF], f32, name=f"bc_{nm}")
        for nm in ["hWu_b16", "lWu_b16", "cb16"]:
            bc[nm] = biasp.tile([P, HALF], fp16, name=f"bc_{nm}")
        # [P,2] const [+0.5, -0.5] for alpha/beta build
        cb2 = wts.tile([P, 2], f32)
        nc.vector.memset(cb2[:, 0:1], 0.5)
        nc.vector.memset(cb2[:, 1:2], -0.5)

        # ================= input pool + prefetch =================
        inp = ctx.enter_context(tc.tile_pool(name="inp", bufs=2))
        prefetched = {}
        for i in range(min(2, n_tiles)):
            for nm, wdt in (("hl", HALF), ("hr", HALF), ("xh", HALF),
                            ("xw", DIM)):
                t = inp.tile([P, wdt], f32, tag=nm, name=f"pre_{nm}_{i}")
                nc.sync.dma_start(t, d[nm][bass.ts(i, P), :])
                prefetched[(i, nm)] = t

        # ---------------- one-time setup ----------------
        with tc.tile_pool(name="setup", bufs=1) as sp, \
                tc.tile_pool(name="psum_setup", bufs=2, space="PSUM") as pss:

            def bias_row(nm):
                r = sp.tile([1, HALF], f32, tag="row", bufs=2, name=f"row_{nm}")
                nc.sync.dma_start(r, d[nm][None, :])
                return r

            def bcast(dst, row_ap):
                nc.gpsimd.partition_broadcast(dst, row_ap)

            def tmp_bc(nm, row_ap):
                t = sp.tile([P, HALF], f32, tag="tbc", bufs=5, name=f"tbc_{nm}")
                bcast(t, row_ap)
                return t

            # A-side biases straight to persistent f32 broadcasts
            bcast(bc["qWu_b"], bias_row("qWu_b"))
            bcast(bc["kWu_b"], bias_row("kWu_b"))

            qUb_bc = tmp_bc("qU_b", bias_row("qU_b"))
            lUb_bc = tmp_bc("lU_b", bias_row("lU_b"))
            qWub_bc = bc["qWu_b"]

            ftmp = sp.tile([P, HALF], f32, tag="ftmp")
            # qb' = qWb_b + qU_b*qWu_b
            bcast(bc["qb"], bias_row("qWb_b"))
            nc.vector.tensor_mul(ftmp, qUb_bc, qWub_bc)
            nc.vector.tensor_add(bc["qb"], bc["qb"], ftmp)

            def wtemp(w, nch):
                t = sp.tile([P, nch, HALF], f32, tag=f"wtmp{nch}",
                            bufs=(4 if nch == 4 else 1), name=f"wtmp_{w}")
                rr = d[w + "_w"].rearrange("(c p) o -> p c o", p=P)
                for c in range(nch):
                    nc.sync.dma_start(t[:, c, :], rr[:, c, :])
                return t

            # A-phase weights -> fp16
            for w in ["qU", "kU", "kWu"]:
                t = wtemp(w, 4)
                for c in range(4):
                    nc.vector.tensor_copy(wsb[w][:, c, :], t[:, c, :])
            # qWb' = qWb + qWu*diag(qU_b)  (column scale via broadcast)
            qWu_tmp = wtemp("qWu", 4)
            qWb_tmp = wtemp("qWb", 4)
            for c in range(4):
                nc.vector.tensor_copy(wsb["qWu"][:, c, :], qWu_tmp[:, c, :])
                nc.vector.tensor_mul(ftmp, qWu_tmp[:, c, :], qUb_bc)
                nc.vector.tensor_add(ftmp, qWb_tmp[:, c, :], ftmp)
                nc.vector.tensor_copy(wsb["qWb"][:, c, :], ftmp)

            # hU: alpha-fold, cs/bh rows, WS/WT/WB
            hU_tmp = wtemp("hU", 8)
            bh_ps = pss.tile([1, HALF], f32)
            cs_ps = pss.tile([1, HALF], f32)
            for c in range(8):
                nc.tensor.matmul(bh_ps, beta_sb[:, c:c + 1], hU_tmp[:, c, :],
                                 start=(c == 0), stop=(c == 7))
            for c in range(8):
                nc.tensor.matmul(cs_ps, alpha_sb[:, c:c + 1], hU_tmp[:, c, :],
                                 start=(c == 0), stop=(c == 7))
            bh_row = sp.tile([1, HALF], f32, tag="row", bufs=2)
            nc.vector.tensor_add(bh_row, bh_ps, bias_row("hU_b"))
            cs_row = sp.tile([1, HALF], f32, tag="row", bufs=2)
            nc.vector.tensor_copy(cs_row, cs_ps)
            bcast(bc["cs"], cs_row)
            bh_bc = tmp_bc("bh", bh_row)

            # hU_a rows scaled by alpha; WT=top, WB=bot, WS=top+bot (fp16)
            hUa = sp.tile([P, 8, HALF], f32, tag="hUa", bufs=1)
            for c in range(8):
                nc.vector.tensor_scalar_mul(hUa[:, c, :], hU_tmp[:, c, :],
                                            alpha_sb[:, c:c + 1])
            for c in range(4):
                nc.vector.tensor_copy(wsb["WT"][:, c, :], hUa[:, c, :])
                nc.vector.tensor_copy(wsb["WB"][:, c, :], hUa[:, 4 + c, :])
                nc.vector.tensor_add(ftmp, hUa[:, c, :], hUa[:, 4 + c, :])
                nc.vector.tensor_copy(wsb["WS"][:, c, :], ftmp)

            lU_tmp = wtemp("lU", 8)
            for c in range(8):
                nc.vector.tensor_copy(wsb["lU"][:, c, :], lU_tmp[:, c, :])

            # hidden/leaf hyper biases -> fp16 broadcasts
            hWub_bc = tmp_bc("hWu_b", bias_row("hWu_b"))
            lWub_bc = tmp_bc("lWu_b", bias_row("lWu_b"))
            nc.vector.tensor_copy(bc["hWu_b16"], hWub_bc)
            nc.vector.tensor_copy(bc["lWu_b16"], lWub_bc)

            # cb = hWb_b + bh*hWu_b + lWb_b + lU_b*lWu_b  (fp16 broadcast)
            cbf = sp.tile([P, HALF], f32, tag="cbf")
            bcast(cbf, bias_row("hWb_b"))
            nc.vector.tensor_mul(ftmp, bh_bc, hWub_bc)
            nc.vector.tensor_add(cbf, cbf, ftmp)
            lWbb_bc = tmp_bc("lWb_b", bias_row("lWb_b"))
            nc.vector.tensor_add(cbf, cbf, lWbb_bc)
            nc.vector.tensor_mul(ftmp, lUb_bc, lWub_bc)
            nc.vector.tensor_add(cbf, cbf, ftmp)
            nc.vector.tensor_copy(bc["cb16"], cbf)

            # WC = hWb + hWu*diag(bh) + lWb + lWu*diag(lU_b)
            hWu_tmp = wtemp("hWu", 4)
            lWu_tmp = wtemp("lWu", 4)
            hWb_tmp = wtemp("hWb", 4)
            lWb_tmp = wtemp("lWb", 4)
            for c in range(4):
                nc.vector.tensor_copy(wsb["hWu"][:, c, :], hWu_tmp[:, c, :])
                nc.vector.tensor_copy(wsb["lWu"][:, c, :], lWu_tmp[:, c, :])
                nc.vector.tensor_mul(ftmp, hWu_tmp[:, c, :], bh_bc)
                nc.vector.tensor_add(hWb_tmp[:, c, :], hWb_tmp[:, c, :], ftmp)
                nc.vector.tensor_add(hWb_tmp[:, c, :], hWb_tmp[:, c, :],
                                     lWb_tmp[:, c, :])
                nc.vector.tensor_mul(ftmp, lWu_tmp[:, c, :], lUb_bc)
                nc.vector.tensor_add(ftmp, hWb_tmp[:, c, :], ftmp)
                nc.vector.tensor_copy(wsb["WC"][:, c, :], ftmp)

        # ================= main loop pools =================
        b16 = ctx.enter_context(tc.tile_pool(name="b16", bufs=2))
        tsp = ctx.enter_context(tc.tile_pool(name="tsp", bufs=2))
        scr = ctx.enter_context(tc.tile_pool(name="scr", bufs=3))
        att = ctx.enter_context(tc.tile_pool(name="att", bufs=2))
        tinyp = ctx.enter_context(tc.tile_pool(name="tinyp", bufs=2))
        phd = ctx.enter_context(tc.tile_pool(name="phd", bufs=2))
        outp = ctx.enter_context(tc.tile_pool(name="outp", bufs=2))
        tp_ps = ctx.enter_context(tc.tile_pool(name="tp_ps", bufs=2,
                                               space="PSUM"))
        mm_ps = ctx.enter_context(tc.tile_pool(name="mm_ps", bufs=6,
                                               space="PSUM"))

        for i in range(n_tiles):
            rs = bass.ts(i, P)
            # ---- loads ----
            if (i, "hl") in prefetched:
                hl_t = prefetched.pop((i, "hl"))
                hr_t = prefetched.pop((i, "hr"))
                xh_t = prefetched.pop((i, "xh"))
                xw_t = prefetched.pop((i, "xw"))
            else:
                hl_t = inp.tile([P, HALF], f32, tag="hl")
                nc.sync.dma_start(hl_t, d["hl"][rs, :])
                hr_t = inp.tile([P, HALF], f32, tag="hr")
                nc.sync.dma_start(hr_t, d["hr"][rs, :])
                xh_t = inp.tile([P, HALF], f32, tag="xh")
                nc.sync.dma_start(xh_t, d["xh"][rs, :])
                xw_t = inp.tile([P, DIM], f32, tag="xw")
                nc.sync.dma_start(xw_t, d["xw"][rs, :])

            # ---- basis build + stats accums ----
            stats = tinyp.tile([P, 8], f32, tag="stats")
            sh, sd = stats[:, 3:4], stats[:, 4:5]
            qh, qd = stats[:, 5:6], stats[:, 6:7]
            chd = tinyp.tile([P, 1], f32, tag="chd")

            hs_b = b16.tile([P, HALF], fp16, tag="hs")
            nc.vector.scalar_tensor_tensor(hs_b, hl_t, 0.0, hr_t,
                                           ALU.bypass, ALU.add, accum_out=sh)
            hd_b = b16.tile([P, HALF], fp16, tag="hd")
            nc.vector.scalar_tensor_tensor(hd_b, hl_t, 0.0, hr_t,
                                           ALU.bypass, ALU.subtract,
                                           accum_out=sd)
            xh_b = b16.tile([P, HALF], fp16, tag="xhb")
            nc.scalar.copy(xh_b, xh_t)
            xw_b = b16.tile([P, DIM], fp16, tag="xwb")
            nc.scalar.copy(xw_b[:, :HALF], xw_t[:, :HALF])
            nc.scalar.copy(xw_b[:, HALF:], xw_t[:, HALF:])

            s1g = scr.tile([P, HALF], fp16, tag="scr", name=f"scr_qh_{i}")
            nc.vector.scalar_tensor_tensor(s1g, hs_b, 0.0, hs_b,
                                           ALU.bypass, ALU.mult, accum_out=qh)
            s2g = scr.tile([P, HALF], fp16, tag="scr", name=f"scr_qd_{i}")
            nc.vector.scalar_tensor_tensor(s2g, hd_b, 0.0, hd_b,
                                           ALU.bypass, ALU.mult, accum_out=qd)
            s3g = scr.tile([P, HALF], fp16, tag="scr", name=f"scr_chd_{i}")
            nc.vector.scalar_tensor_tensor(s3g, hs_b, 0.0, hd_b,
                                           ALU.bypass, ALU.mult,
                                           accum_out=chd)

            # ---- PE transposes (feature-major stationaries) ----
            def transpose_to(src, ncols, tg):
                sb = tsp.tile([P, ncols * P], fp16, tag=tg, name=f"T_{tg}_{i}")
                for g in range(0, ncols, 4):
                    ps = tp_ps.tile([P, 4 * P], fp16, tag="tp",
                                    name=f"tps_{tg}_{g}_{i}")
                    gw = min(4, ncols - g)
                    for c in range(gw):
                        nc.tensor.transpose(
                            ps[:, c * P:(c + 1) * P],
                            src[:, (g + c) * P:(g + c + 1) * P],
                            ident)
                    nc.scalar.copy(sb[:, g * P:(g + gw) * P], ps[:, :gw * P])
                return sb

            hsT = transpose_to(hs_b, 4, "ThS")
            hdT = transpose_to(hd_b, 4, "ThD")
            xhT = transpose_to(xh_b, 4, "TxH")
            xwT = transpose_to(xw_b, 8, "TxW")

            # ---- phase A matmuls ----
            def unit(tag):
                return mm_ps.tile([P, HALF], f32, tag="mm", name=f"ps_{tag}_{i}")

            SUq, SBq, TU = unit("SUq"), unit("SBq"), unit("TU")
            for c in range(4):
                lhs = xhT[:, bass.ts(c, P)]
                st, sp_ = (c == 0), (c == 3)
                nc.tensor.matmul(SUq, lhs, wsb["qWu"][:, c, :], start=st, stop=sp_)
                nc.tensor.matmul(SBq, lhs, wsb["qWb"][:, c, :], start=st, stop=sp_)
                nc.tensor.matmul(TU, lhs, wsb["kWu"][:, c, :], start=st, stop=sp_)
            CD = unit("CD")
            for c in range(4):
                nc.tensor.matmul(CD, hdT[:, bass.ts(c, P)], wsb["kU"][:, c, :],
                                 start=(c == 0), stop=(c == 3))
            AS = unit("AS")
            for c in range(4):
                nc.tensor.matmul(AS, hsT[:, bass.ts(c, P)], wsb["qU"][:, c, :],
                                 start=(c == 0), stop=(c == 3))
            AD = unit("AD")
            for c in range(4):
                nc.tensor.matmul(AD, hdT[:, bass.ts(c, P)], wsb["qU"][:, c, :],
                                 start=(c == 0), stop=(c == 3))

            # ---- phase A elementwise ----
            su = att.tile([P, HALF], fp16, tag="su")
            nc.vector.tensor_add(su, SUq, bc["qWu_b"])
            sbq = att.tile([P, HALF], fp16, tag="sbq")
            nc.vector.tensor_add(sbq, SBq, bc["qb"])
            tu = att.tile([P, HALF], fp16, tag="tu")
            nc.vector.tensor_add(tu, TU, bc["kWu_b"])
            dk = att.tile([P, HALF], fp16, tag="dk")
            nc.vector.tensor_mul(dk, CD, tu)
            # u = su*dk (the (AS±AD)/2 halving folds into the dd combine)
            u = att.tile([P, HALF], fp16, tag="u")
            nc.gpsimd.tensor_mul(u, su, dk)

            for j, (aa, bb) in enumerate([(sbq, dk), (AS, u), (AD, u)]):
                sdot = scr.tile([P, HALF], fp16, tag="scr_b",
                                name=f"scr_dot{j}_{i}")
                nc.vector.scalar_tensor_tensor(
                    sdot, aa, 0.0, bb, ALU.bypass, ALU.mult,
                    accum_out=stats[:, j:j + 1])

            # ---- phase D matmuls (attention-independent) ----
            Mb = unit("Mb")
            for c in range(4):
                nc.tensor.matmul(Mb, hsT[:, bass.ts(c, P)], wsb["WS"][:, c, :],
                                 start=(c == 0), stop=(c == 3))
            D1 = unit("D1")
            for c in range(4):
                nc.tensor.matmul(D1, hdT[:, bass.ts(c, P)], wsb["WT"][:, c, :],
                                 start=(c == 0), stop=(c == 3))
            D2 = unit("D2")
            for c in range(4):
                nc.tensor.matmul(D2, hdT[:, bass.ts(c, P)], wsb["WB"][:, c, :],
                                 start=(c == 0), stop=(c == 3))
            # hyper units with identity-folded bias rows
            HSU, LSU, SBC = unit("HSU"), unit("LSU"), unit("SBC")
            for c in range(4):
                lhs = xhT[:, bass.ts(c, P)]
                st = (c == 0)
                nc.tensor.matmul(HSU, lhs, wsb["hWu"][:, c, :], start=st, stop=False)
                nc.tensor.matmul(LSU, lhs, wsb["lWu"][:, c, :], start=st, stop=False)
                nc.tensor.matmul(SBC, lhs, wsb["WC"][:, c, :], start=st, stop=False)
            nc.tensor.matmul(HSU, ident, bc["hWu_b16"], start=False, stop=True)
            nc.tensor.matmul(LSU, ident, bc["lWu_b16"], start=False, stop=True)
            nc.tensor.matmul(SBC, ident, bc["cb16"], start=False, stop=True)
            LUp = unit("LU")
            for c in range(8):
                nc.tensor.matmul(LUp, xwT[:, bass.ts(c, P)], wsb["lU"][:, c, :],
                                 start=(c == 0), stop=(c == 7))

            # ---- tiny chain: d0/d1, sigmoid, layernorm stats ----
            ee = tinyp.tile([P, 2], f32, tag="ee")
            nc.vector.tensor_add(ee[:, 0:1], stats[:, 1:2], stats[:, 2:3])
            nc.vector.scalar_tensor_tensor(ee[:, 1:2], stats[:, 2:3], -1.0,
                                           stats[:, 1:2], ALU.mult, ALU.add)
            dd = tinyp.tile([P, 2], f32, tag="dd")
            nc.vector.scalar_tensor_tensor(
                dd, ee, 0.5, stats[:, 0:1].broadcast_to([P, 2]),
                ALU.mult, ALU.add)
            z2 = tinyp.tile([P, 2], f32, tag="z2")
            nc.scalar.activation(z2, dd, ACTF.Square, scale=IS)
            sq1 = tinyp.tile([P, 2], f32, tag="sq1")
            nc.scalar.activation(sq1, z2, ACTF.Sqrt, scale=1.0 / SIG_C1,
                                 bias=1.0)
            sq2 = tinyp.tile([P, 2], f32, tag="sq2")
            nc.scalar.activation(sq2, z2, ACTF.Sqrt, scale=1.0 / SIG_C2,
                                 bias=1.0)
            r1 = tinyp.tile([P, 2], f32, tag="r1")
            nc.vector.reciprocal(r1, sq1)
            r2 = tinyp.tile([P, 2], f32, tag="r2")
            nc.vector.reciprocal(r2, sq2)
            mm_ = tinyp.tile([P, 2], f32, tag="mm2")
            nc.vector.scalar_tensor_tensor(mm_, r1, SIG_K1 / SIG_K2, r2,
                                           ALU.mult, ALU.add)
            psh = tinyp.tile([P, 2], f32, tag="psh")
            nc.vector.scalar_tensor_tensor(psh, dd, SIG_K2, mm_,
                                           ALU.mult, ALU.mult)
            ab = tinyp.tile([P, 2], f32, tag="ab")
            nc.vector.tensor_add(ab, psh, cb2)
            al, be = ab[:, 0:1], ab[:, 1:2]

            g_t = tinyp.tile([P, 4], f32, tag="gt")
            gg, m2t, st_t, ssqh = (g_t[:, 0:1], g_t[:, 1:2], g_t[:, 2:3],
                                   g_t[:, 3:4])
            nc.vector.tensor_add(gg, al, be)
            sqab = tinyp.tile([P, 2], f32, tag="sqab")
            nc.vector.tensor_mul(sqab, ab, ab)
            dl_t = tinyp.tile([P, 4], f32, tag="dlt")
            dl, gh, dlh, sumxh = (dl_t[:, 0:1], dl_t[:, 1:2], dl_t[:, 2:3],
                                  dl_t[:, 3:4])
            nc.vector.tensor_add(dl, sqab[:, 0:1], sqab[:, 1:2])
            nc.scalar.activation(gh, gg, ACTF.Copy, scale=0.5)
            nc.scalar.activation(dlh, dl, ACTF.Copy, scale=0.5)
            # sumx/2 = sh + 0.5*g*sd
            nc.vector.scalar_tensor_tensor(sumxh, sd, gh, sh, ALU.mult, ALU.add)
            # ssq/2 = qh + g*chd + 0.5*dl*qd
            nc.vector.scalar_tensor_tensor(st_t, chd, gg, qh, ALU.mult, ALU.add)
            nc.vector.scalar_tensor_tensor(ssqh, qd, dlh, st_t, ALU.mult,
                                           ALU.add)
            nc.vector.tensor_mul(m2t, sumxh, sumxh)
            varh = tinyp.tile([P, 4], f32, tag="varh")
            nc.vector.scalar_tensor_tensor(varh[:, 0:1], m2t, -1.0 / 512.0,
                                           ssqh, ALU.mult, ALU.add)
            # std = sqrt(2/1023 * varh)
            nc.scalar.activation(varh[:, 1:2], varh[:, 0:1], ACTF.Sqrt,
                                 scale=2.0 / (DIM - 1))
            nc.vector.reciprocal(varh[:, 2:3], varh[:, 1:2])
            nc.scalar.activation(varh[:, 3:4], varh[:, 2:3], ACTF.Copy,
                                 scale=-1.0)
            nrinv = varh[:, 3:4]
            mean = tinyp.tile([P, 1], f32, tag="mean")
            nc.scalar.activation(mean, sumxh, ACTF.Copy, scale=1.0 / 512.0)

            # ---- hidden path combine ----
            # (DVE may read only one PSUM operand per op: evict Mb first)
            mb_sb = phd.tile([P, HALF], f32, tag="mb_sb")
            nc.scalar.copy(mb_sb, Mb)
            t_h = phd.tile([P, HALF], f32, tag="t_h")
            nc.vector.scalar_tensor_tensor(t_h, D1, al, mb_sb, ALU.mult,
                                           ALU.add)
            hu = phd.tile([P, HALF], f32, tag="hu")
            nc.vector.scalar_tensor_tensor(hu, D2, be, t_h, ALU.mult, ALU.add)
            t5 = phd.tile([P, HALF], fp16, tag="t5")
            nc.vector.scalar_tensor_tensor(t5, bc["cs"], mean, hu,
                                           ALU.mult, ALU.subtract)
            u1 = phd.tile([P, HALF], fp16, tag="u1")
            nc.scalar.activation(u1, t5, ACTF.Copy, scale=nrinv)

            # evict hyper results (biases already folded in PSUM)
            su_h = phd.tile([P, HALF], fp16, tag="su_h")
            nc.scalar.copy(su_h, HSU)
            su_l = phd.tile([P, HALF], fp16, tag="su_l")
            nc.scalar.copy(su_l, LSU)
            sbc = phd.tile([P, HALF], fp16, tag="sbc")
            nc.scalar.copy(sbc, SBC)

            w1 = phd.tile([P, HALF], fp16, tag="w1")
            nc.vector.tensor_mul(w1, LUp, su_l)
            v1 = phd.tile([P, HALF], fp16, tag="v1")
            nc.gpsimd.tensor_mul(v1, u1, su_h)
            s2 = phd.tile([P, HALF], fp16, tag="s2")
            nc.gpsimd.tensor_add(s2, v1, sbc)
            out_t = outp.tile([P, HALF], f32, tag="out_t")
            nc.gpsimd.tensor_add(out_t, s2, w1)

            nc.sync.dma_start(out_d[rs, :], out_t)

    nc.compile()
    return nc


_NC_CACHE = {}


def _get_nc(b_loc, mm_dt=None):
    key = b_loc
    if key not in _NC_CACHE:
        _NC_CACHE[key] = build_nc(b_loc)
    return _NC_CACHE[key]


def kernel(**inputs):
    b = inputs["hl"].shape[0]
    n_cores = N_CORES
    b_loc = b // n_cores
    nc = _get_nc(b_loc)

    sharded = {"hl", "hr", "xw", "xh"}
    in_maps = []
    for i in range(n_cores):
        m = {}
        for k, v in inputs.items():
            v = np.ascontiguousarray(np.asarray(v, dtype=np.float32))
            if k in sharded:
                m[k] = v[i * b_loc:(i + 1) * b_loc]
            else:
                m[k] = v
        in_maps.append(m)

    res = run_bass_kernel_spmd(nc, in_maps, core_ids=list(range(n_cores)))
    return np.concatenate([r["out"] for r in res.results], axis=0)


# revision 14
# speedup vs baseline: 1.0447x; 1.0447x over previous
"""Trainium2 Bass kernel for nn_ChildHAggregation (gnn_message_passing).

Per-sample math (B=32768, HALF=512, DIM=1024):
  x = [hl, hr]; 2-token attention with HyperLinear q/k; layernorm;
  out = hidden(x_norm, xh) + leaf(xw, xh)   (both HyperLinear)

v3 design, pure data-parallel, batch-major [128 x feat] tiles:
  - ALL weight folding is done host-side in numpy (fp16, pre-rearranged
    for contiguous DMA); the device program has no setup compute.
  - hs/hd basis: hs=hl+hr, hd=hl-hr built in TRANSPOSED space from
    hlT/hrT; layernorm stats derived from ql/qr/cr2 row accumulations.
  - score diffs d0/d1 via the difference trick; p00/p11 via two-term
    softsign sigmoid (max err 1.9e-3) using only Square/Sqrt/reciprocal.
  - M-path is attention-free: x@hU_a = hs@WS + p00*(hd@WT) - p11*(hd@WB)
  - output hyper biases folded into PSUM by an identity-stationary
    matmul of the broadcast bias tile.
  - emission is software-pipelined: tile i+1's input casts are emitted
    before tile i's tail so no engine queue stalls the next tile's PE.
All matmul operands fp16 (same PE speed as bf16, 8x finer mantissa).
"""

from contextlib import ExitStack

import numpy as np

import concourse.bacc as bacc
import concourse.bass as bass
import concourse.mybir as mybir
import concourse.tile as tile
from concourse.bass_utils import run_bass_kernel_spmd
from concourse.masks import make_identity

N_CORES = 8
HALF = 512
DIM = 1024
P = 128
IS = 1.0 / float(np.sqrt(np.float32(HALF)))

# two-term softsign sigmoid constants (max |err| 1.9e-3 over |z|<=14)
SIG_A1 = 2.057838
SIG_C1 = 8.347378
SIG_A2 = 0.5 - SIG_A1
SIG_C2 = 11.527823
SIG_K1 = SIG_A1 * IS / float(np.sqrt(SIG_C1))
SIG_K2 = SIG_A2 * IS / float(np.sqrt(SIG_C2))

f32 = mybir.dt.float32
fp16 = mybir.dt.float16

ALU = mybir.AluOpType
ACTF = mybir.ActivationFunctionType

W4 = ["qU3", "kU3", "qWu3", "qWbF", "kWu3", "WS", "WT", "WB",
      "hWu3", "lWu3", "WC3"]
BCN = ["b_qWu", "b_kWu", "b_qb", "b_cs", "b_hWu", "b_lWu", "b_cb"]


def _r4(w):
    """[512, 512] -> [128, 4, 512] chunk-major fp16, contiguous."""
    return np.ascontiguousarray(
        w.reshape(4, P, HALF).transpose(1, 0, 2).astype(np.float16))


def _r8(w):
    return np.ascontiguousarray(
        w.reshape(8, P, HALF).transpose(1, 0, 2).astype(np.float16))


def _bc(row):
    return np.ascontiguousarray(
        np.broadcast_to(row.astype(np.float16)[None, :], (P, HALF)))


def host_prep(inputs):
    """Fold weights/biases in f32 numpy; emit fp16 device buffers."""
    g = {k: np.asarray(v, dtype=np.float32) for k, v in inputs.items()}
    out = {}
    out["qU3"] = _r4(g["qU_w"])
    out["kU3"] = _r4(g["kU_w"])
    out["qWu3"] = _r4(g["qWu_w"])
    out["kWu3"] = _r4(g["kWu_w"])
    out["qWbF"] = _r4(g["qWb_w"] + g["qWu_w"] * g["qU_b"][None, :])
    hU_a = g["hU_w"] * g["alpha"][:, None]
    out["WS"] = _r4(hU_a[:HALF] + hU_a[HALF:])
    out["WT"] = _r4(hU_a[:HALF])
    out["WB"] = _r4(hU_a[HALF:])
    out["hWu3"] = _r4(g["hWu_w"])
    out["lWu3"] = _r4(g["lWu_w"])
    bh = g["beta"] @ g["hU_w"] + g["hU_b"]
    out["WC3"] = _r4(g["hWb_w"] + g["hWu_w"] * bh[None, :]
                     + g["lWb_w"] + g["lWu_w"] * g["lU_b"][None, :])
    out["lU3"] = _r8(g["lU_w"])
    out["b_qWu"] = _bc(g["qWu_b"])
    out["b_kWu"] = _bc(g["kWu_b"])
    out["b_qb"] = _bc(g["qWb_b"] + g["qU_b"] * g["qWu_b"])
    out["b_cs"] = _bc(g["alpha"] @ g["hU_w"])
    out["b_hWu"] = _bc(g["hWu_b"])
    out["b_lWu"] = _bc(g["lWu_b"])
    out["b_cb"] = _bc(g["hWb_b"] + bh * g["hWu_b"]
                      + g["lWb_b"] + g["lU_b"] * g["lWu_b"])
    return out


def build_nc(b_loc):
    n_tiles = b_loc // P
    assert n_tiles * P == b_loc

    nc = bacc.Bacc("TRN2", target_bir_lowering=False, debug=False,
                   num_devices=1)

    d = {}
    d["hl"] = nc.dram_tensor("hl", [b_loc, HALF], f32, kind="ExternalInput").ap()
    d["hr"] = nc.dram_tensor("hr", [b_loc, HALF], f32, kind="ExternalInput").ap()
    d["xw"] = nc.dram_tensor("xw", [b_loc, DIM], f32, kind="ExternalInput").ap()
    d["xh"] = nc.dram_tensor("xh", [b_loc, HALF], f32, kind="ExternalInput").ap()
    for w in W4:
        d[w] = nc.dram_tensor(w, [P, 4, HALF], fp16, kind="ExternalInput").ap()
    d["lU3"] = nc.dram_tensor("lU3", [P, 8, HALF], fp16,
                              kind="ExternalInput").ap()
    for w in BCN:
        d[w] = nc.dram_tensor(w, [P, HALF], fp16, kind="ExternalInput").ap()
    out_d = nc.dram_tensor("out", [b_loc, HALF], f32, kind="ExternalOutput").ap()

    with tile.TileContext(nc) as tc, ExitStack() as ctx:
        wts = ctx.enter_context(tc.tile_pool(name="wts", bufs=1))
        wsb = {}
        for w in W4:
            wsb[w] = wts.tile([P, 4, HALF], fp16, name=f"w_{w}")
            nc.sync.dma_start(wsb[w], d[w])
        wsb["lU3"] = wts.tile([P, 8, HALF], fp16, name="w_lU3")
        nc.sync.dma_start(wsb["lU3"], d["lU3"])
        bc = {}
        for w in BCN:
            bc[w] = wts.tile([P, HALF], fp16, name=f"bc_{w}")
            nc.sync.dma_start(bc[w], d[w])
        ident = wts.tile([P, P], fp16)
        make_identity(nc, ident)
        cb2 = wts.tile([P, 2], f32)
        nc.vector.memset(cb2[:, 0:1], 0.5)
        nc.vector.memset(cb2[:, 1:2], -0.5)

        inp = ctx.enter_context(tc.tile_pool(name="inp", bufs=3))
        b16 = ctx.enter_context(tc.tile_pool(name="b16", bufs=3))
        tsp = ctx.enter_context(tc.tile_pool(name="tsp", bufs=2))
        scr = ctx.enter_context(tc.tile_pool(name="scr", bufs=3))
        att = ctx.enter_context(tc.tile_pool(name="att", bufs=2))
        tinyp = ctx.enter_context(tc.tile_pool(name="tinyp", bufs=3))
        phd = ctx.enter_context(tc.tile_pool(name="phd", bufs=2))
        outp = ctx.enter_context(tc.tile_pool(name="outp", bufs=2))
        tp_ps = ctx.enter_context(tc.tile_pool(name="tp_ps", bufs=2,
                                               space="PSUM"))
        mm_ps = ctx.enter_context(tc.tile_pool(name="mm_ps", bufs=6,
                                               space="PSUM"))

        st0 = {}

        def stage0(i):
            """Loads + input downcasts (+ sl/sr accums) for tile i."""
            rs = bass.ts(i, P)
            hl_t = inp.tile([P, HALF], f32, tag="hl", name=f"hl_{i}")
            nc.sync.dma_start(hl_t, d["hl"][rs, :])
            hr_t = inp.tile([P, HALF], f32, tag="hr", name=f"hr_{i}")
            nc.sync.dma_start(hr_t, d["hr"][rs, :])
            xh_t = inp.tile([P, HALF], f32, tag="xh", name=f"xh_{i}")
            nc.sync.dma_start(xh_t, d["xh"][rs, :])
            xw_t = inp.tile([P, DIM], f32, tag="xw", name=f"xw_{i}")
            nc.sync.dma_start(xw_t, d["xw"][rs, :])

            sS = tinyp.tile([P, 2], f32, tag="sS", name=f"sS_{i}")
            hl_b = b16.tile([P, HALF], fp16, tag="hlb", name=f"hlb_{i}")
            nc.scalar.activation(hl_b, hl_t, ACTF.Copy, accum_out=sS[:, 0:1])
            hr_b = b16.tile([P, HALF], fp16, tag="hrb", name=f"hrb_{i}")
            nc.scalar.activation(hr_b, hr_t, ACTF.Copy, accum_out=sS[:, 1:2])
            xh_b = b16.tile([P, HALF], fp16, tag="xhb", name=f"xhb_{i}")
            nc.scalar.copy(xh_b, xh_t)
            xw_b = b16.tile([P, DIM], fp16, tag="xwb", name=f"xwb_{i}")
            nc.gpsimd.tensor_copy(xw_b[:, :HALF], xw_t[:, :HALF])
            nc.gpsimd.tensor_copy(xw_b[:, HALF:], xw_t[:, HALF:])
            st0[i] = (hl_t, hr_t, xh_t, xw_t, hl_b, hr_b, xh_b, xw_b, sS)

        def unit(tag, i):
            return mm_ps.tile([P, HALF], f32, tag="mm", name=f"ps_{tag}_{i}")

        def transpose_to(src, ncols, tg, i):
            sb = tsp.tile([P, ncols * P], fp16, tag=tg, name=f"T_{tg}_{i}")
            for g in range(0, ncols, 4):
                ps = tp_ps.tile([P, 4 * P], fp16, tag="tp",
                                name=f"tps_{tg}_{g}_{i}")
                gw = min(4, ncols - g)
                for c in range(gw):
                    nc.tensor.transpose(ps[:, c * P:(c + 1) * P],
                                        src[:, (g + c) * P:(g + c + 1) * P],
                                        ident)
                nc.scalar.copy(sb[:, g * P:(g + gw) * P], ps[:, :gw * P])
            return sb

        def stage1a(i):
            """Transposes, basis build, A-phase + D-phase matmuls."""
            (hl_t, hr_t, xh_t, xw_t, hl_b, hr_b, xh_b, xw_b, sS) = st0[i]

            hlT = transpose_to(hl_b, 4, "ThL", i)
            hrT = transpose_to(hr_b, 4, "ThR", i)
            hsT = tsp.tile([P, 4 * P], fp16, tag="ThS", name=f"T_ThS_{i}")
            nc.vector.tensor_add(hsT, hlT, hrT)
            hdT = tsp.tile([P, 4 * P], fp16, tag="ThD", name=f"T_ThD_{i}")
            nc.vector.tensor_sub(hdT, hlT, hrT)
            xhT = transpose_to(xh_b, 4, "TxH", i)
            xwT = transpose_to(xw_b, 8, "TxW", i)

            # row stats from f32 inputs (only need the DMA)
            qS = tinyp.tile([P, 8], f32, tag="qS", name=f"qS_{i}")
            sg1 = scr.tile([P, HALF], fp16, tag="scr", name=f"scr_ql_{i}")
            nc.vector.scalar_tensor_tensor(sg1, hl_t, 0.0, hl_t, ALU.bypass,
                                           ALU.mult, accum_out=qS[:, 0:1])
            sg2 = scr.tile([P, HALF], fp16, tag="scr", name=f"scr_qr_{i}")
            nc.vector.scalar_tensor_tensor(sg2, hr_t, 0.0, hr_t, ALU.bypass,
                                           ALU.mult, accum_out=qS[:, 1:2])
            sg3 = scr.tile([P, HALF], fp16, tag="scr", name=f"scr_cr_{i}")
            nc.vector.scalar_tensor_tensor(sg3, hl_t, 0.0, hr_t, ALU.bypass,
                                           ALU.mult, accum_out=qS[:, 2:3])

            # ---- A-phase matmuls ----
            SUq, SBq, TU = unit("SUq", i), unit("SBq", i), unit("TU", i)
            for c in range(4):
                lhs = xhT[:, bass.ts(c, P)]
                st, sp_ = (c == 0), (c == 3)
                nc.tensor.matmul(SUq, lhs, wsb["qWu3"][:, c, :], start=st, stop=sp_)
                nc.tensor.matmul(SBq, lhs, wsb["qWbF"][:, c, :], start=st, stop=sp_)
                nc.tensor.matmul(TU, lhs, wsb["kWu3"][:, c, :], start=st, stop=sp_)
            CD = unit("CD", i)
            for c in range(4):
                nc.tensor.matmul(CD, hdT[:, bass.ts(c, P)],
                                 wsb["kU3"][:, c, :], start=(c == 0),
                                 stop=(c == 3))
            AS = unit("AS", i)
            for c in range(4):
                nc.tensor.matmul(AS, hsT[:, bass.ts(c, P)],
                                 wsb["qU3"][:, c, :], start=(c == 0),
                                 stop=(c == 3))
            AD = unit("AD", i)
            for c in range(4):
                nc.tensor.matmul(AD, hdT[:, bass.ts(c, P)],
                                 wsb["qU3"][:, c, :], start=(c == 0),
                                 stop=(c == 3))

            # ---- A-phase elementwise ----
            su = att.tile([P, HALF], fp16, tag="su", name=f"su_{i}")
            nc.vector.tensor_add(su, SUq, bc["b_qWu"])
            sbq = att.tile([P, HALF], fp16, tag="sbq", name=f"sbq_{i}")
            nc.vector.tensor_add(sbq, SBq, bc["b_qb"])
            tu = att.tile([P, HALF], fp16, tag="tu", name=f"tu_{i}")
            nc.vector.tensor_add(tu, TU, bc["b_kWu"])
            dk = att.tile([P, HALF], fp16, tag="dk", name=f"dk_{i}")
            nc.vector.tensor_mul(dk, CD, tu)
            u = att.tile([P, HALF], fp16, tag="u", name=f"u_{i}")
            nc.gpsimd.tensor_mul(u, su, dk)

            for j, (aa, bb) in enumerate([(sbq, dk), (AS, u), (AD, u)]):
                sdot = scr.tile([P, HALF], fp16, tag="scr_b",
                                name=f"scr_dot{j}_{i}")
                nc.vector.scalar_tensor_tensor(
                    sdot, aa, 0.0, bb, ALU.bypass, ALU.mult,
                    accum_out=qS[:, 3 + j:4 + j])

            # ---- D-phase matmuls (attention-independent) ----
            HSU, LSU = unit("HSU", i), unit("LSU", i)
            SBC = unit("SBC", i)
            for c in range(4):
                lhs = xhT[:, bass.ts(c, P)]
                st = (c == 0)
                nc.tensor.matmul(HSU, lhs, wsb["hWu3"][:, c, :], start=st,
                                 stop=False)
                nc.tensor.matmul(LSU, lhs, wsb["lWu3"][:, c, :], start=st,
                                 stop=False)
                nc.tensor.matmul(SBC, lhs, wsb["WC3"][:, c, :], start=st,
                                 stop=False)
            nc.tensor.matmul(HSU, ident, bc["b_hWu"], start=False, stop=True)
            nc.tensor.matmul(LSU, ident, bc["b_lWu"], start=False, stop=True)
            nc.tensor.matmul(SBC, ident, bc["b_cb"], start=False, stop=True)
            LUp = unit("LU", i)
            for c in range(8):
                nc.tensor.matmul(LUp, xwT[:, bass.ts(c, P)],
                                 wsb["lU3"][:, c, :], start=(c == 0),
                                 stop=(c == 7))
            Mb = unit("Mb", i)
            for c in range(4):
                nc.tensor.matmul(Mb, hsT[:, bass.ts(c, P)],
                                 wsb["WS"][:, c, :], start=(c == 0),
                                 stop=(c == 3))
            D1 = unit("D1", i)
            for c in range(4):
                nc.tensor.matmul(D1, hdT[:, bass.ts(c, P)],
                                 wsb["WT"][:, c, :], start=(c == 0),
                                 stop=(c == 3))
            D2 = unit("D2", i)
            for c in range(4):
                nc.tensor.matmul(D2, hdT[:, bass.ts(c, P)],
                                 wsb["WB"][:, c, :], start=(c == 0),
                                 stop=(c == 3))
            return (qS, sS, HSU, LSU, SBC, LUp, Mb, D1, D2)

        def stage1b(i, h):
            (qS, sS, HSU, LSU, SBC, LUp, Mb, D1, D2) = h
            ql, qr, cr2 = qS[:, 0:1], qS[:, 1:2], qS[:, 2:3]
            cdt, sA, dA = qS[:, 3:4], qS[:, 4:5], qS[:, 5:6]
            sl, sr = sS[:, 0:1], sS[:, 1:2]

            # d0/d1 and two-term softsign sigmoid -> ab = [p00, -p11]
            ee = tinyp.tile([P, 2], f32, tag="ee", name=f"ee_{i}")
            nc.vector.tensor_add(ee[:, 0:1], sA, dA)
            nc.vector.scalar_tensor_tensor(ee[:, 1:2], dA, -1.0, sA,
                                           ALU.mult, ALU.add)
            dd = tinyp.tile([P, 2], f32, tag="dd", name=f"dd_{i}")
            nc.vector.scalar_tensor_tensor(dd, ee, 0.5,
                                           cdt.broadcast_to([P, 2]),
                                           ALU.mult, ALU.add)
            z2 = tinyp.tile([P, 2], f32, tag="z2", name=f"z2_{i}")
            nc.scalar.activation(z2, dd, ACTF.Square, scale=IS)
            sq1 = tinyp.tile([P, 2], f32, tag="sq1", name=f"sq1_{i}")
            nc.scalar.activation(sq1, z2, ACTF.Sqrt, scale=1.0 / SIG_C1,
                                 bias=1.0)
            sq2 = tinyp.tile([P, 2], f32, tag="sq2", name=f"sq2_{i}")
            nc.scalar.activation(sq2, z2, ACTF.Sqrt, scale=1.0 / SIG_C2,
                                 bias=1.0)
            rr = tinyp.tile([P, 4], f32, tag="rr", name=f"rr_{i}")
            nc.vector.reciprocal(rr[:, 0:2], sq1)
            nc.vector.reciprocal(rr[:, 2:4], sq2)
            mm_ = tinyp.tile([P, 2], f32, tag="mm2", name=f"mm2_{i}")
            nc.vector.scalar_tensor_tensor(mm_, rr[:, 0:2], SIG_K1 / SIG_K2,
                                           rr[:, 2:4], ALU.mult, ALU.add)
            psh = tinyp.tile([P, 2], f32, tag="psh", name=f"psh_{i}")
            nc.vector.scalar_tensor_tensor(psh, dd, SIG_K2, mm_,
                                           ALU.mult, ALU.mult)
            ab = tinyp.tile([P, 2], f32, tag="ab", name=f"ab_{i}")
            nc.vector.tensor_add(ab, psh, cb2)
            al, be = ab[:, 0:1], ab[:, 1:2]

            # layernorm stats:
            #   qh=ql+qr+2cr2, qd=ql+qr-2cr2, chd=ql-qr
            #   ssq/2 = ql*(1+g+dl/2) + qr*(1-g+dl/2) + cr2*(2-dl)
            #   sumx/2 = (sl+sr) + 0.5g*(sl-sr)
            gt = tinyp.tile([P, 8], f32, tag="gt", name=f"gt_{i}")
            g_, gh, dl, base = gt[:, 0:1], gt[:, 1:2], gt[:, 2:3], gt[:, 3:4]
            cA, cB, cC = gt[:, 4:5], gt[:, 5:6], gt[:, 6:7]
            nc.vector.tensor_add(g_, al, be)
            nc.vector.tensor_scalar(gh, g_, 0.5, None, ALU.mult)
            sqab = tinyp.tile([P, 2], f32, tag="sqab", name=f"sqab_{i}")
            nc.vector.tensor_mul(sqab, ab, ab)
            nc.vector.tensor_add(dl, sqab[:, 0:1], sqab[:, 1:2])
            nc.vector.tensor_scalar(base, dl, 0.5, 1.0, ALU.mult, ALU.add)
            nc.vector.tensor_add(cA, base, g_)
            nc.vector.scalar_tensor_tensor(cB, g_, -1.0, base, ALU.mult,
                                           ALU.add)
            nc.vector.tensor_scalar(cC, dl, -1.0, 2.0, ALU.mult, ALU.add)
            acc = tinyp.tile([P, 8], f32, tag="acc", name=f"acc_{i}")
            z0, z1, ssqh = acc[:, 0:1], acc[:, 1:2], acc[:, 2:3]
            sh2, sd2, sumxh = acc[:, 3:4], acc[:, 4:5], acc[:, 5:6]
            m2, varh = acc[:, 6:7], acc[:, 7:8]
            nc.vector.tensor_scalar_mul(z0, cr2, cC)
            nc.vector.scalar_tensor_tensor(z1, ql, cA, z0, ALU.mult, ALU.add)
            nc.vector.scalar_tensor_tensor(ssqh, qr, cB, z1, ALU.mult,
                                           ALU.add)
            nc.vector.tensor_add(sh2, sl, sr)
            nc.vector.tensor_sub(sd2, sl, sr)
            nc.vector.scalar_tensor_tensor(sumxh, sd2, gh, sh2, ALU.mult,
                                           ALU.add)
            nc.vector.tensor_mul(m2, sumxh, sumxh)
            nc.vector.scalar_tensor_tensor(varh, m2, -1.0 / 512.0, ssqh,
                                           ALU.mult, ALU.add)
            so = tinyp.tile([P, 4], f32, tag="so", name=f"so_{i}")
            sqstd, rinv, nrinv, mean = (so[:, 0:1], so[:, 1:2], so[:, 2:3],
                                        so[:, 3:4])
            nc.scalar.activation(sqstd, varh, ACTF.Sqrt,
                                 scale=2.0 / (DIM - 1))
            nc.vector.reciprocal(rinv, sqstd)
            nc.scalar.activation(nrinv, rinv, ACTF.Copy, scale=-1.0)
            nc.scalar.activation(mean, sumxh, ACTF.Copy, scale=1.0 / 512.0)

            # ---- hidden path: drain D1/D2 on scalar, combine on vector ----
            th0 = phd.tile([P, HALF], fp16, tag="th0", name=f"th0_{i}")
            nc.scalar.activation(th0, D1, ACTF.Copy, scale=al)
            th1 = phd.tile([P, HALF], fp16, tag="th1", name=f"th1_{i}")
            nc.scalar.activation(th1, D2, ACTF.Copy, scale=be)
            hv = phd.tile([P, HALF], fp16, tag="hv", name=f"hv_{i}")
            nc.vector.tensor_add(hv, th0, th1)
            t5a = phd.tile([P, HALF], f32, tag="t5a", name=f"t5a_{i}")
            nc.vector.scalar_tensor_tensor(t5a, bc["b_cs"], mean, Mb,
                                           ALU.mult, ALU.subtract)
            t5 = phd.tile([P, HALF], fp16, tag="t5", name=f"t5_{i}")
            nc.vector.tensor_sub(t5, t5a, hv)
            u1 = phd.tile([P, HALF], fp16, tag="u1", name=f"u1_{i}")
            nc.scalar.activation(u1, t5, ACTF.Copy, scale=nrinv)

            su_h = phd.tile([P, HALF], fp16, tag="su_h", name=f"su_h_{i}")
            nc.scalar.copy(su_h, HSU)
            su_l = phd.tile([P, HALF], fp16, tag="su_l", name=f"su_l_{i}")
            nc.scalar.copy(su_l, LSU)
            sbc = phd.tile([P, HALF], fp16, tag="sbc", name=f"sbc_{i}")
            nc.scalar.copy(sbc, SBC)

            w1 = phd.tile([P, HALF], fp16, tag="w1", name=f"w1_{i}")
            nc.vector.tensor_mul(w1, LUp, su_l)
            v1 = phd.tile([P, HALF], fp16, tag="v1", name=f"v1_{i}")
            nc.gpsimd.tensor_mul(v1, u1, su_h)
            s2 = phd.tile([P, HALF], fp16, tag="s2", name=f"s2_{i}")
            nc.gpsimd.tensor_add(s2, v1, sbc)
            out_t = outp.tile([P, HALF], f32, tag="out_t", name=f"out_{i}")
            nc.gpsimd.tensor_add(out_t, s2, w1)
            nc.sync.dma_start(out_d[bass.ts(i, P), :], out_t)

        stage0(0)
        for i in range(n_tiles):
            h = stage1a(i)
            if i + 1 < n_tiles:
                stage0(i + 1)
            stage1b(i, h)

    nc.compile()
    return nc


_NC_CACHE = {}


def _get_nc(b_loc, mm_dt=None):
    if b_loc not in _NC_CACHE:
        _NC_CACHE[b_loc] = build_nc(b_loc)
    return _NC_CACHE[b_loc]


def make_in_maps(inputs):
    b = inputs["hl"].shape[0]
    b_loc = b // N_CORES
    prep = host_prep(inputs)
    in_maps = []
    for i in range(N_CORES):
        m = {}
        for k in ("hl", "hr", "xw", "xh"):
            v = np.ascontiguousarray(np.asarray(inputs[k], dtype=np.float32))
            m[k] = v[i * b_loc:(i + 1) * b_loc]
        m.update(prep)
        in_maps.append(m)
    return in_maps


def kernel(**inputs):
    b = inputs["hl"].shape[0]
    nc = _get_nc(b // N_CORES)
    in_maps = make_in_maps(inputs)
    res = run_bass_kernel_spmd(nc, in_maps, core_ids=list(range(N_CORES)))
    return np.concatenate([r["out"] for r in res.results], axis=0)


# revision 16
# speedup vs baseline: 1.1537x; 1.1043x over previous
"""Trainium2 Bass kernel for nn_ChildHAggregation (gnn_message_passing).

Per-sample math (B=32768, HALF=512, DIM=1024):
  x = [hl, hr]; 2-token attention with HyperLinear q/k; layernorm;
  out = hidden(x_norm, xh) + leaf(xw, xh)   (both HyperLinear)

v4 design, pure data-parallel, batch-major [128 x feat] tiles:
  - ALL weight folding is host-side numpy (fp16, pre-rearranged for
    contiguous DMA); no device-side setup compute.
  - hs/hd basis (hs=hl+hr, hd=hl-hr) built in TRANSPOSED space;
    layernorm stats derived from ql/qr/cr2 row accumulations.
  - d0/d1 via the score-difference trick; p00/p11 via two-term softsign
    sigmoid (max err 1.9e-3) using only Square/Sqrt/reciprocal.
  - M-path is attention-free: x@hU_a = hs@WS + p00*(hd@WT) - p11*(hd@WB)
  - xw is transposed in f32 on the PE (2 cycles/row) and downcast in the
    PSUM eviction - no separate xw cast op.
  - 3-stage software pipeline: tile i+1's loads/casts AND transposes are
    emitted before tile i's tail so no engine queue blocks the PE.
  - D-phase PSUM unit order chosen so the 6-bank rotation always reuses
    a bank whose consumer ran early (su_h/su_l/sbc/w1 head of the tail;
    Mb scalar-evicted right after the sigmoid).
All matmul operands fp16 (same PE speed as bf16, 8x finer mantissa).
"""

from contextlib import ExitStack

import numpy as np

import concourse.bacc as bacc
import concourse.bass as bass
import concourse.mybir as mybir
import concourse.tile as tile
from concourse.bass_utils import run_bass_kernel_spmd
from concourse.masks import make_identity

N_CORES = 8
HALF = 512
DIM = 1024
P = 128
IS = 1.0 / float(np.sqrt(np.float32(HALF)))

# two-term softsign sigmoid constants (max |err| 1.9e-3 over |z|<=14)
SIG_A1 = 2.057838
SIG_C1 = 8.347378
SIG_A2 = 0.5 - SIG_A1
SIG_C2 = 11.527823
SIG_K1 = SIG_A1 * IS / float(np.sqrt(SIG_C1))
SIG_K2 = SIG_A2 * IS / float(np.sqrt(SIG_C2))

f32 = mybir.dt.float32
fp16 = mybir.dt.float16

ALU = mybir.AluOpType
ACTF = mybir.ActivationFunctionType

W4 = ["qU3", "kU3", "qWu3", "qWbF", "kWu3", "WS", "WT", "WB",
      "hWu3", "lWu3", "WC3"]
BCN = ["b_qWu", "b_kWu", "b_qb", "b_cs512", "b_hWu", "b_lWu", "b_cb"]


def _r4(w):
    return np.ascontiguousarray(
        w.reshape(4, P, HALF).transpose(1, 0, 2).astype(np.float16))


def _r8(w):
    return np.ascontiguousarray(
        w.reshape(8, P, HALF).transpose(1, 0, 2).astype(np.float16))


def _bc(row):
    return np.ascontiguousarray(
        np.broadcast_to(row.astype(np.float16)[None, :], (P, HALF)))


def host_prep(inputs):
    """Fold weights/biases in f32 numpy; emit fp16 device buffers."""
    g = {k: np.asarray(v, dtype=np.float32) for k, v in inputs.items()}
    out = {}
    out["qU3"] = _r4(g["qU_w"])
    out["kU3"] = _r4(g["kU_w"])
    out["qWu3"] = _r4(g["qWu_w"])
    out["kWu3"] = _r4(g["kWu_w"])
    out["qWbF"] = _r4(g["qWb_w"] + g["qWu_w"] * g["qU_b"][None, :])
    hU_a = g["hU_w"] * g["alpha"][:, None]
    out["WS"] = _r4(hU_a[:HALF] + hU_a[HALF:])
    out["WT"] = _r4(hU_a[:HALF])
    out["WB"] = _r4(hU_a[HALF:])
    out["hWu3"] = _r4(g["hWu_w"])
    out["lWu3"] = _r4(g["lWu_w"])
    bh = g["beta"] @ g["hU_w"] + g["hU_b"]
    out["WC3"] = _r4(g["hWb_w"] + g["hWu_w"] * bh[None, :]
                     + g["lWb_w"] + g["lWu_w"] * g["lU_b"][None, :])
    out["lU3"] = _r8(g["lU_w"])
    out["b_qWu"] = _bc(g["qWu_b"])
    out["b_kWu"] = _bc(g["kWu_b"])
    out["b_qb"] = _bc(g["qWb_b"] + g["qU_b"] * g["qWu_b"])
    out["b_cs512"] = _bc((g["alpha"] @ g["hU_w"]) / 512.0)
    out["b_hWu"] = _bc(g["hWu_b"])
    out["b_lWu"] = _bc(g["lWu_b"])
    out["b_cb"] = _bc(g["hWb_b"] + bh * g["hWu_b"]
                      + g["lWb_b"] + g["lU_b"] * g["lWu_b"])
    return out


def build_nc(b_loc):
    n_tiles = b_loc // P
    assert n_tiles * P == b_loc

    nc = bacc.Bacc("TRN2", target_bir_lowering=False, debug=False,
                   num_devices=1)

    d = {}
    d["hl"] = nc.dram_tensor("hl", [b_loc, HALF], f32, kind="ExternalInput").ap()
    d["hr"] = nc.dram_tensor("hr", [b_loc, HALF], f32, kind="ExternalInput").ap()
    d["xw"] = nc.dram_tensor("xw", [b_loc, DIM], f32, kind="ExternalInput").ap()
    d["xh"] = nc.dram_tensor("xh", [b_loc, HALF], f32, kind="ExternalInput").ap()
    for w in W4:
        d[w] = nc.dram_tensor(w, [P, 4, HALF], fp16, kind="ExternalInput").ap()
    d["lU3"] = nc.dram_tensor("lU3", [P, 8, HALF], fp16,
                              kind="ExternalInput").ap()
    for w in BCN:
        d[w] = nc.dram_tensor(w, [P, HALF], fp16, kind="ExternalInput").ap()
    out_d = nc.dram_tensor("out", [b_loc, HALF], f32, kind="ExternalOutput").ap()

    with tile.TileContext(nc) as tc, ExitStack() as ctx:
        wts = ctx.enter_context(tc.tile_pool(name="wts", bufs=1))
        wsb = {}
        for w in W4:
            wsb[w] = wts.tile([P, 4, HALF], fp16, name=f"w_{w}")
            nc.sync.dma_start(wsb[w], d[w])
        wsb["lU3"] = wts.tile([P, 8, HALF], fp16, name="w_lU3")
        nc.sync.dma_start(wsb["lU3"], d["lU3"])
        bc = {}
        for w in BCN:
            bc[w] = wts.tile([P, HALF], fp16, name=f"bc_{w}")
            nc.sync.dma_start(bc[w], d[w])
        ident = wts.tile([P, P], fp16)
        make_identity(nc, ident)
        ident32 = wts.tile([P, P], f32)
        make_identity(nc, ident32)
        cb2 = wts.tile([P, 2], f32)
        nc.vector.memset(cb2[:, 0:1], 0.5)
        nc.vector.memset(cb2[:, 1:2], -0.5)

        inp = ctx.enter_context(tc.tile_pool(name="inp", bufs=3))
        b16 = ctx.enter_context(tc.tile_pool(name="b16", bufs=3))
        tsp = ctx.enter_context(tc.tile_pool(name="tsp", bufs=2))
        scr = ctx.enter_context(tc.tile_pool(name="scr", bufs=3))
        att = ctx.enter_context(tc.tile_pool(name="att", bufs=2))
        tinyp = ctx.enter_context(tc.tile_pool(name="tinyp", bufs=3))
        phd = ctx.enter_context(tc.tile_pool(name="phd", bufs=2))
        outp = ctx.enter_context(tc.tile_pool(name="outp", bufs=2))
        tp_ps = ctx.enter_context(tc.tile_pool(name="tp_ps", bufs=2,
                                               space="PSUM"))
        mm_ps = ctx.enter_context(tc.tile_pool(name="mm_ps", bufs=6,
                                               space="PSUM"))

        st0 = {}
        stT = {}

        def stage0(i):
            """Loads + hl/hr/xh downcasts (+ sl/sr accums)."""
            rs = bass.ts(i, P)
            hl_t = inp.tile([P, HALF], f32, tag="hl", name=f"hl_{i}")
            nc.sync.dma_start(hl_t, d["hl"][rs, :])
            hr_t = inp.tile([P, HALF], f32, tag="hr", name=f"hr_{i}")
            nc.sync.dma_start(hr_t, d["hr"][rs, :])
            xh_t = inp.tile([P, HALF], f32, tag="xh", name=f"xh_{i}")
            nc.sync.dma_start(xh_t, d["xh"][rs, :])
            xw_t = inp.tile([P, DIM], f32, tag="xw", name=f"xw_{i}")
            nc.sync.dma_start(xw_t, d["xw"][rs, :])

            sS = tinyp.tile([P, 2], f32, tag="sS", name=f"sS_{i}")
            hl_b = b16.tile([P, HALF], fp16, tag="hlb", name=f"hlb_{i}")
            nc.scalar.activation(hl_b, hl_t, ACTF.Copy, accum_out=sS[:, 0:1])
            hr_b = b16.tile([P, HALF], fp16, tag="hrb", name=f"hrb_{i}")
            nc.scalar.activation(hr_b, hr_t, ACTF.Copy, accum_out=sS[:, 1:2])
            xh_b = b16.tile([P, HALF], fp16, tag="xhb", name=f"xhb_{i}")
            nc.scalar.copy(xh_b, xh_t)
            st0[i] = (hl_t, hr_t, xh_t, xw_t, hl_b, hr_b, xh_b, sS)

        def stageT(i):
            """PE transposes + scalar evicts + hs/hd basis (vector)."""
            (hl_t, hr_t, xh_t, xw_t, hl_b, hr_b, xh_b, sS) = st0[i]

            def tp16(src, tg):
                sb = tsp.tile([P, 4 * P], fp16, tag=tg, name=f"T_{tg}_{i}")
                ps = tp_ps.tile([P, 4 * P], fp16, tag="tp",
                                name=f"tps_{tg}_{i}")
                for c in range(4):
                    nc.tensor.transpose(ps[:, c * P:(c + 1) * P],
                                        src[:, c * P:(c + 1) * P], ident)
                nc.scalar.copy(sb, ps)
                return sb

            hlT = tp16(hl_b, "ThL")
            hrT = tp16(hr_b, "ThR")
            hsT = tsp.tile([P, 4 * P], fp16, tag="ThS", name=f"T_ThS_{i}")
            nc.vector.tensor_add(hsT, hlT, hrT)
            hdT = tsp.tile([P, 4 * P], fp16, tag="ThD", name=f"T_ThD_{i}")
            nc.vector.tensor_sub(hdT, hlT, hrT)
            xhT = tp16(xh_b, "TxH")
            # xw: f32 transpose, downcast in the eviction.  Half-width
            # f32 PSUM groups (1KB) share the fp16 groups' pool slot.
            xwT = tsp.tile([P, 8 * P], fp16, tag="TxW", name=f"T_TxW_{i}")
            for g in range(4):
                ps = tp_ps.tile([P, 2 * P], f32, tag="tp",
                                name=f"tpw_{g}_{i}")
                for c in range(2):
                    nc.tensor.transpose(
                        ps[:, c * P:(c + 1) * P],
                        xw_t[:, (2 * g + c) * P:(2 * g + c + 1) * P], ident32)
                nc.scalar.copy(xwT[:, g * 2 * P:(g + 1) * 2 * P], ps)
            stT[i] = (hsT, hdT, xhT, xwT)

        def unit(tag, i):
            return mm_ps.tile([P, HALF], f32, tag="mm", name=f"ps_{tag}_{i}")

        def stage1m(i):
            """A+D matmuls and A-phase elementwise."""
            (hl_t, hr_t, xh_t, xw_t, hl_b, hr_b, xh_b, sS) = st0[i]
            (hsT, hdT, xhT, xwT) = stT[i]

            qS = tinyp.tile([P, 8], f32, tag="qS", name=f"qS_{i}")
            # ql/qr on scalar (Square + accum), cr2 on vector
            sg1 = scr.tile([P, HALF], fp16, tag="scr", name=f"scr_ql_{i}")
            nc.scalar.activation(sg1, hl_t, ACTF.Square,
                                 accum_out=qS[:, 0:1])
            sg2 = scr.tile([P, HALF], fp16, tag="scr", name=f"scr_qr_{i}")
            nc.scalar.activation(sg2, hr_t, ACTF.Square,
                                 accum_out=qS[:, 1:2])
            sg3 = scr.tile([P, HALF], fp16, tag="scr", name=f"scr_cr_{i}")
            nc.vector.scalar_tensor_tensor(sg3, hl_t, 0.0, hr_t, ALU.bypass,
                                           ALU.mult, accum_out=qS[:, 2:3])

            SUq, SBq, TU = unit("SUq", i), unit("SBq", i), unit("TU", i)
            for c in range(4):
                lhs = xhT[:, bass.ts(c, P)]
                st, sp_ = (c == 0), (c == 3)
                nc.tensor.matmul(SUq, lhs, wsb["qWu3"][:, c, :], start=st, stop=sp_)
                nc.tensor.matmul(SBq, lhs, wsb["qWbF"][:, c, :], start=st, stop=sp_)
                nc.tensor.matmul(TU, lhs, wsb["kWu3"][:, c, :], start=st, stop=sp_)
            CD = unit("CD", i)
            for c in range(4):
                nc.tensor.matmul(CD, hdT[:, bass.ts(c, P)],
                                 wsb["kU3"][:, c, :], start=(c == 0),
                                 stop=(c == 3))
            AS = unit("AS", i)
            for c in range(4):
                nc.tensor.matmul(AS, hsT[:, bass.ts(c, P)],
                                 wsb["qU3"][:, c, :], start=(c == 0),
                                 stop=(c == 3))
            AD = unit("AD", i)
            for c in range(4):
                nc.tensor.matmul(AD, hdT[:, bass.ts(c, P)],
                                 wsb["qU3"][:, c, :], start=(c == 0),
                                 stop=(c == 3))

            su = att.tile([P, HALF], fp16, tag="su", name=f"su_{i}")
            nc.vector.tensor_add(su, SUq, bc["b_qWu"])
            sbq = att.tile([P, HALF], fp16, tag="sbq", name=f"sbq_{i}")
            nc.vector.tensor_add(sbq, SBq, bc["b_qb"])
            tu = att.tile([P, HALF], fp16, tag="tu", name=f"tu_{i}")
            nc.vector.tensor_add(tu, TU, bc["b_kWu"])
            dk = att.tile([P, HALF], fp16, tag="dk", name=f"dk_{i}")
            nc.vector.tensor_mul(dk, CD, tu)
            u = att.tile([P, HALF], fp16, tag="u", name=f"u_{i}")
            nc.gpsimd.tensor_mul(u, su, dk)

            for j, (aa, bb) in enumerate([(sbq, dk), (AS, u), (AD, u)]):
                sdot = scr.tile([P, HALF], fp16, tag="scr_b",
                                name=f"scr_dot{j}_{i}")
                nc.vector.scalar_tensor_tensor(
                    sdot, aa, 0.0, bb, ALU.bypass, ALU.mult,
                    accum_out=qS[:, 3 + j:4 + j])

            # D-phase (attention-independent). Unit order tuned for the
            # 6-slot PSUM rotation: each alloc reuses a bank whose
            # consumer runs early.
            HSU, LSU = unit("HSU", i), unit("LSU", i)
            SBC = unit("SBC", i)
            for c in range(4):
                lhs = xhT[:, bass.ts(c, P)]
                st, sp_ = (c == 0), (c == 3)
                nc.tensor.matmul(HSU, lhs, wsb["hWu3"][:, c, :], start=st, stop=sp_)
                nc.tensor.matmul(LSU, lhs, wsb["lWu3"][:, c, :], start=st, stop=sp_)
                nc.tensor.matmul(SBC, lhs, wsb["WC3"][:, c, :], start=st, stop=sp_)
            LUp = unit("LU", i)
            for c in range(8):
                nc.tensor.matmul(LUp, xwT[:, bass.ts(c, P)],
                                 wsb["lU3"][:, c, :], start=(c == 0),
                                 stop=(c == 7))
            Mb = unit("Mb", i)
            for c in range(4):
                nc.tensor.matmul(Mb, hsT[:, bass.ts(c, P)],
                                 wsb["WS"][:, c, :], start=(c == 0),
                                 stop=(c == 3))
            D1 = unit("D1", i)
            for c in range(4):
                nc.tensor.matmul(D1, hdT[:, bass.ts(c, P)],
                                 wsb["WT"][:, c, :], start=(c == 0),
                                 stop=(c == 3))
            D2 = unit("D2", i)
            for c in range(4):
                nc.tensor.matmul(D2, hdT[:, bass.ts(c, P)],
                                 wsb["WB"][:, c, :], start=(c == 0),
                                 stop=(c == 3))
            return (qS, sS, HSU, LSU, SBC, LUp, Mb, D1, D2)

        def stage1b(i, h):
            (qS, sS, HSU, LSU, SBC, LUp, Mb, D1, D2) = h
            ql, qr, cr2 = qS[:, 0:1], qS[:, 1:2], qS[:, 2:3]
            cdt, sA, dA = qS[:, 3:4], qS[:, 4:5], qS[:, 5:6]
            sl, sr = sS[:, 0:1], sS[:, 1:2]

            # vector head: drain D-phase PSUM banks fast
            su_h = phd.tile([P, HALF], fp16, tag="su_h", name=f"su_h_{i}")
            nc.vector.tensor_add(su_h, HSU, bc["b_hWu"])
            su_l = phd.tile([P, HALF], fp16, tag="su_l", name=f"su_l_{i}")
            nc.vector.tensor_add(su_l, LSU, bc["b_lWu"])
            sbc = phd.tile([P, HALF], fp16, tag="sbc", name=f"sbc_{i}")
            nc.vector.tensor_add(sbc, SBC, bc["b_cb"])
            w1 = phd.tile([P, HALF], fp16, tag="w1", name=f"w1_{i}")
            nc.vector.tensor_mul(w1, LUp, su_l)
            # Mb eviction on scalar (frees its bank early; also needed
            # because DVE reads at most one PSUM operand per op)
            mb_sb = phd.tile([P, HALF], f32, tag="mb_sb", name=f"mb_{i}")
            nc.scalar.copy(mb_sb, Mb)

            # d0/d1 + two-term softsign sigmoid -> ab = [p00, -p11]
            ee = tinyp.tile([P, 2], f32, tag="ee", name=f"ee_{i}")
            nc.vector.tensor_add(ee[:, 0:1], sA, dA)
            nc.vector.scalar_tensor_tensor(ee[:, 1:2], dA, -1.0, sA,
                                           ALU.mult, ALU.add)
            dd = tinyp.tile([P, 2], f32, tag="dd", name=f"dd_{i}")
            nc.vector.scalar_tensor_tensor(dd, ee, 0.5,
                                           cdt.broadcast_to([P, 2]),
                                           ALU.mult, ALU.add)
            z2 = tinyp.tile([P, 2], f32, tag="z2", name=f"z2_{i}")
            nc.scalar.activation(z2, dd, ACTF.Square, scale=IS)
            sq1 = tinyp.tile([P, 2], f32, tag="sq1", name=f"sq1_{i}")
            nc.scalar.activation(sq1, z2, ACTF.Sqrt, scale=1.0 / SIG_C1,
                                 bias=1.0)
            sq2 = tinyp.tile([P, 2], f32, tag="sq2", name=f"sq2_{i}")
            nc.scalar.activation(sq2, z2, ACTF.Sqrt, scale=1.0 / SIG_C2,
                                 bias=1.0)
            rr = tinyp.tile([P, 4], f32, tag="rr", name=f"rr_{i}")
            nc.vector.reciprocal(rr[:, 0:2], sq1)
            nc.vector.reciprocal(rr[:, 2:4], sq2)
            mm_ = tinyp.tile([P, 2], f32, tag="mm2", name=f"mm2_{i}")
            nc.vector.scalar_tensor_tensor(mm_, rr[:, 0:2], SIG_K1 / SIG_K2,
                                           rr[:, 2:4], ALU.mult, ALU.add)
            psh = tinyp.tile([P, 2], f32, tag="psh", name=f"psh_{i}")
            nc.vector.scalar_tensor_tensor(psh, dd, SIG_K2, mm_,
                                           ALU.mult, ALU.mult)
            ab = tinyp.tile([P, 2], f32, tag="ab", name=f"ab_{i}")
            nc.vector.tensor_add(ab, psh, cb2)
            al, be = ab[:, 0:1], ab[:, 1:2]

            # D1/D2 drained by scalar as soon as al/be exist
            th0 = phd.tile([P, HALF], fp16, tag="th0", name=f"th0_{i}")
            nc.scalar.activation(th0, D1, ACTF.Copy, scale=al)
            th1 = phd.tile([P, HALF], fp16, tag="th1", name=f"th1_{i}")
            nc.scalar.activation(th1, D2, ACTF.Copy, scale=be)
            hv = phd.tile([P, HALF], fp16, tag="hv", name=f"hv_{i}")
            nc.gpsimd.tensor_add(hv, th0, th1)

            # layernorm stats
            gt = tinyp.tile([P, 8], f32, tag="gt", name=f"gt_{i}")
            g_, gh, dl, base = gt[:, 0:1], gt[:, 1:2], gt[:, 2:3], gt[:, 3:4]
            cA, cB, cC = gt[:, 4:5], gt[:, 5:6], gt[:, 6:7]
            nc.vector.tensor_add(g_, al, be)
            nc.vector.tensor_scalar(gh, g_, 0.5, None, ALU.mult)
            sqab = tinyp.tile([P, 2], f32, tag="sqab", name=f"sqab_{i}")
            nc.vector.tensor_mul(sqab, ab, ab)
            nc.vector.tensor_add(dl, sqab[:, 0:1], sqab[:, 1:2])
            nc.vector.tensor_scalar(base, dl, 0.5, 1.0, ALU.mult, ALU.add)
            nc.vector.tensor_add(cA, base, g_)
            nc.vector.scalar_tensor_tensor(cB, g_, -1.0, base, ALU.mult,
                                           ALU.add)
            nc.vector.tensor_scalar(cC, dl, -1.0, 2.0, ALU.mult, ALU.add)
            acc = tinyp.tile([P, 8], f32, tag="acc", name=f"acc_{i}")
            z0, z1, ssqh = acc[:, 0:1], acc[:, 1:2], acc[:, 2:3]
            sh2, sd2, sumxh = acc[:, 3:4], acc[:, 4:5], acc[:, 5:6]
            m2, varh = acc[:, 6:7], acc[:, 7:8]
            nc.vector.tensor_scalar_mul(z0, cr2, cC)
            nc.vector.scalar_tensor_tensor(z1, ql, cA, z0, ALU.mult, ALU.add)
            nc.vector.scalar_tensor_tensor(ssqh, qr, cB, z1, ALU.mult,
                                           ALU.add)
            nc.vector.tensor_add(sh2, sl, sr)
            nc.vector.tensor_sub(sd2, sl, sr)
            nc.vector.scalar_tensor_tensor(sumxh, sd2, gh, sh2, ALU.mult,
                                           ALU.add)
            nc.vector.tensor_mul(m2, sumxh, sumxh)
            nc.vector.scalar_tensor_tensor(varh, m2, -1.0 / 512.0, ssqh,
                                           ALU.mult, ALU.add)
            so = tinyp.tile([P, 2], f32, tag="so", name=f"so_{i}")
            sqstd, rinv = so[:, 0:1], so[:, 1:2]
            nc.scalar.activation(sqstd, varh, ACTF.Sqrt,
                                 scale=2.0 / (DIM - 1))
            nc.vector.reciprocal(rinv, sqstd)

            # t5 = (Mb + hv) - cs*mean ; u1 = rinv * t5
            t5a = phd.tile([P, HALF], f32, tag="t5a", name=f"t5a_{i}")
            nc.vector.scalar_tensor_tensor(t5a, bc["b_cs512"], sumxh, mb_sb,
                                           ALU.mult, ALU.subtract)
            t5 = phd.tile([P, HALF], fp16, tag="t5", name=f"t5_{i}")
            nc.vector.tensor_sub(t5, hv, t5a)
            u1 = phd.tile([P, HALF], fp16, tag="u1", name=f"u1_{i}")
            nc.scalar.activation(u1, t5, ACTF.Copy, scale=rinv)

            v1 = phd.tile([P, HALF], fp16, tag="v1", name=f"v1_{i}")
            nc.gpsimd.tensor_mul(v1, u1, su_h)
            s2 = phd.tile([P, HALF], fp16, tag="s2", name=f"s2_{i}")
            nc.gpsimd.tensor_add(s2, v1, sbc)
            out_t = outp.tile([P, HALF], f32, tag="out_t", name=f"out_{i}")
            nc.gpsimd.tensor_add(out_t, s2, w1)
            nc.sync.dma_start(out_d[bass.ts(i, P), :], out_t)

        stage0(0)
        stageT(0)
        for i in range(n_tiles):
            h = stage1m(i)
            if i + 1 < n_tiles:
                stage0(i + 1)
                stageT(i + 1)
            stage1b(i, h)

    nc.compile()
    return nc


_NC_CACHE = {}


def _get_nc(b_loc, mm_dt=None):
    if b_loc not in _NC_CACHE:
        _NC_CACHE[b_loc] = build_nc(b_loc)
    return _NC_CACHE[b_loc]


def make_in_maps(inputs):
    b = inputs["hl"].shape[0]
    b_loc = b // N_CORES
    prep = host_prep(inputs)
    in_maps = []
    for i in range(N_CORES):
        m = {}
        for k in ("hl", "hr", "xw", "xh"):
            v = np.ascontiguousarray(np.asarray(inputs[k], dtype=np.float32))
            m[k] = v[i * b_loc:(i + 1) * b_loc]
        m.update(prep)
        in_maps.append(m)
    return in_maps


def kernel(**inputs):
    b = inputs["hl"].shape[0]
    nc = _get_nc(b // N_CORES)
    in_maps = make_in_maps(inputs)
    res = run_bass_kernel_spmd(nc, in_maps, core_ids=list(range(N_CORES)))
    return np.concatenate([r["out"] for r in res.results], axis=0)


# revision 17
# speedup vs baseline: 1.2043x; 1.0439x over previous
"""Trainium2 Bass kernel for nn_ChildHAggregation (gnn_message_passing).

Per-sample math (B=32768, HALF=512, DIM=1024):
  x = [hl, hr]; 2-token attention with HyperLinear q/k; layernorm;
  out = hidden(x_norm, xh) + leaf(xw, xh)   (both HyperLinear)

v4 design, pure data-parallel, batch-major [128 x feat] tiles:
  - ALL weight folding is host-side numpy (fp16, pre-rearranged for
    contiguous DMA); no device-side setup compute.
  - hs/hd basis (hs=hl+hr, hd=hl-hr) built in TRANSPOSED space;
    layernorm stats derived from ql/qr/cr2 row accumulations.
  - d0/d1 via the score-difference trick; p00/p11 via two-term softsign
    sigmoid (max err 1.9e-3) using only Square/Sqrt/reciprocal.
  - M-path is attention-free: x@hU_a = hs@WS + p00*(hd@WT) - p11*(hd@WB)
  - xw is transposed in f32 on the PE (2 cycles/row) and downcast in the
    PSUM eviction - no separate xw cast op.
  - 3-stage software pipeline: tile i+1's loads/casts AND transposes are
    emitted before tile i's tail so no engine queue blocks the PE.
  - D-phase PSUM unit order chosen so the 6-bank rotation always reuses
    a bank whose consumer ran early (su_h/su_l/sbc/w1 head of the tail;
    Mb scalar-evicted right after the sigmoid).
All matmul operands fp16 (same PE speed as bf16, 8x finer mantissa).
"""

from contextlib import ExitStack

import numpy as np

import concourse.bacc as bacc
import concourse.bass as bass
import concourse.mybir as mybir
import concourse.tile as tile
from concourse.bass_utils import run_bass_kernel_spmd
from concourse.masks import make_identity

N_CORES = 8
HALF = 512
DIM = 1024
P = 128
IS = 1.0 / float(np.sqrt(np.float32(HALF)))

# two-term softsign sigmoid constants (max |err| 1.9e-3 over |z|<=14)
SIG_A1 = 2.057838
SIG_C1 = 8.347378
SIG_A2 = 0.5 - SIG_A1
SIG_C2 = 11.527823
SIG_K1 = SIG_A1 * IS / float(np.sqrt(SIG_C1))
SIG_K2 = SIG_A2 * IS / float(np.sqrt(SIG_C2))

f32 = mybir.dt.float32
fp16 = mybir.dt.float16

ALU = mybir.AluOpType
ACTF = mybir.ActivationFunctionType

W4 = ["qWu3", "qWbF", "kWu3", "kU3", "qU3", "hWu3", "lWu3", "WC3",
      "WS", "WT", "WB"]
BCN = ["b_qWu", "b_kWu", "b_qb", "b_cs512", "b_hWu", "b_lWu", "b_cb"]


def _r4(w):
    return np.ascontiguousarray(
        w.reshape(4, P, HALF).transpose(1, 0, 2).astype(np.float16))


def _r8(w):
    return np.ascontiguousarray(
        w.reshape(8, P, HALF).transpose(1, 0, 2).astype(np.float16))


def _bc(row):
    return np.ascontiguousarray(
        np.broadcast_to(row.astype(np.float16)[None, :], (P, HALF)))


def host_prep(inputs):
    """Fold weights/biases in f32 numpy; emit fp16 device buffers."""
    g = {k: np.asarray(v, dtype=np.float32) for k, v in inputs.items()}
    out = {}
    out["qU3"] = _r4(g["qU_w"])
    out["kU3"] = _r4(g["kU_w"])
    out["qWu3"] = _r4(g["qWu_w"])
    out["kWu3"] = _r4(g["kWu_w"])
    out["qWbF"] = _r4(g["qWb_w"] + g["qWu_w"] * g["qU_b"][None, :])
    hU_a = g["hU_w"] * g["alpha"][:, None]
    out["WS"] = _r4(hU_a[:HALF] + hU_a[HALF:])
    out["WT"] = _r4(hU_a[:HALF])
    out["WB"] = _r4(hU_a[HALF:])
    out["hWu3"] = _r4(g["hWu_w"])
    out["lWu3"] = _r4(g["lWu_w"])
    bh = g["beta"] @ g["hU_w"] + g["hU_b"]
    out["WC3"] = _r4(g["hWb_w"] + g["hWu_w"] * bh[None, :]
                     + g["lWb_w"] + g["lWu_w"] * g["lU_b"][None, :])
    out["lU3"] = _r8(g["lU_w"])
    out["b_qWu"] = _bc(g["qWu_b"])
    out["b_kWu"] = _bc(g["kWu_b"])
    out["b_qb"] = _bc(g["qWb_b"] + g["qU_b"] * g["qWu_b"])
    out["b_cs512"] = _bc((g["alpha"] @ g["hU_w"]) / 512.0)
    out["b_hWu"] = _bc(g["hWu_b"])
    out["b_lWu"] = _bc(g["lWu_b"])
    out["b_cb"] = _bc(g["hWb_b"] + bh * g["hWu_b"]
                      + g["lWb_b"] + g["lU_b"] * g["lWu_b"])
    return out


def build_nc(b_loc):
    n_tiles = b_loc // P
    assert n_tiles * P == b_loc

    nc = bacc.Bacc("TRN2", target_bir_lowering=False, debug=False,
                   num_devices=1)

    d = {}
    d["hl"] = nc.dram_tensor("hl", [b_loc, HALF], f32, kind="ExternalInput").ap()
    d["hr"] = nc.dram_tensor("hr", [b_loc, HALF], f32, kind="ExternalInput").ap()
    d["xw"] = nc.dram_tensor("xw", [b_loc, DIM], f32, kind="ExternalInput").ap()
    d["xh"] = nc.dram_tensor("xh", [b_loc, HALF], f32, kind="ExternalInput").ap()
    for w in W4:
        d[w] = nc.dram_tensor(w, [P, 4, HALF], fp16, kind="ExternalInput").ap()
    d["lU3"] = nc.dram_tensor("lU3", [P, 8, HALF], fp16,
                              kind="ExternalInput").ap()
    for w in BCN:
        d[w] = nc.dram_tensor(w, [P, HALF], fp16, kind="ExternalInput").ap()
    out_d = nc.dram_tensor("out", [b_loc, HALF], f32, kind="ExternalOutput").ap()

    with tile.TileContext(nc) as tc, ExitStack() as ctx:
        wts = ctx.enter_context(tc.tile_pool(name="wts", bufs=1))
        wsb = {}
        for w in W4:
            wsb[w] = wts.tile([P, 4, HALF], fp16, name=f"w_{w}")
            nc.sync.dma_start(wsb[w], d[w])
        wsb["lU3"] = wts.tile([P, 8, HALF], fp16, name="w_lU3")
        nc.sync.dma_start(wsb["lU3"], d["lU3"])
        bc = {}
        for w in BCN:
            bc[w] = wts.tile([P, HALF], fp16, name=f"bc_{w}")
            nc.sync.dma_start(bc[w], d[w])
        ident = wts.tile([P, P], fp16)
        make_identity(nc, ident)
        ident32 = wts.tile([P, P], f32)
        make_identity(nc, ident32)
        cb2 = wts.tile([P, 2], f32)
        nc.vector.memset(cb2[:, 0:1], 0.5)
        nc.vector.memset(cb2[:, 1:2], -0.5)

        inp = ctx.enter_context(tc.tile_pool(name="inp", bufs=4))
        b16 = ctx.enter_context(tc.tile_pool(name="b16", bufs=4))
        tsp = ctx.enter_context(tc.tile_pool(name="tsp", bufs=3))
        scr = ctx.enter_context(tc.tile_pool(name="scr", bufs=3))
        att = ctx.enter_context(tc.tile_pool(name="att", bufs=2))
        tinyp = ctx.enter_context(tc.tile_pool(name="tinyp", bufs=4))
        phd = ctx.enter_context(tc.tile_pool(name="phd", bufs=2))
        outp = ctx.enter_context(tc.tile_pool(name="outp", bufs=2))
        tp_ps = ctx.enter_context(tc.tile_pool(name="tp_ps", bufs=2,
                                               space="PSUM"))
        mm_ps = ctx.enter_context(tc.tile_pool(name="mm_ps", bufs=6,
                                               space="PSUM"))

        st0 = {}
        stT = {}

        def stage0(i):
            """Loads + hl/hr/xh downcasts (+ sl/sr accums)."""
            rs = bass.ts(i, P)
            hl_t = inp.tile([P, HALF], f32, tag="hl", name=f"hl_{i}")
            nc.sync.dma_start(hl_t, d["hl"][rs, :])
            hr_t = inp.tile([P, HALF], f32, tag="hr", name=f"hr_{i}")
            nc.sync.dma_start(hr_t, d["hr"][rs, :])
            xh_t = inp.tile([P, HALF], f32, tag="xh", name=f"xh_{i}")
            nc.sync.dma_start(xh_t, d["xh"][rs, :])
            xw_t = inp.tile([P, DIM], f32, tag="xw", name=f"xw_{i}")
            nc.sync.dma_start(xw_t, d["xw"][rs, :])

            sS = tinyp.tile([P, 2], f32, tag="sS", name=f"sS_{i}")
            hl_b = b16.tile([P, HALF], fp16, tag="hlb", name=f"hlb_{i}")
            nc.scalar.activation(hl_b, hl_t, ACTF.Copy, accum_out=sS[:, 0:1])
            hr_b = b16.tile([P, HALF], fp16, tag="hrb", name=f"hrb_{i}")
            nc.scalar.activation(hr_b, hr_t, ACTF.Copy, accum_out=sS[:, 1:2])
            xh_b = b16.tile([P, HALF], fp16, tag="xhb", name=f"xhb_{i}")
            nc.scalar.copy(xh_b, xh_t)
            st0[i] = (hl_t, hr_t, xh_t, xw_t, hl_b, hr_b, xh_b, sS)

        def stageT(i):
            """PE transposes + scalar evicts + hs/hd basis (vector)."""
            (hl_t, hr_t, xh_t, xw_t, hl_b, hr_b, xh_b, sS) = st0[i]

            def tp16(src, tg):
                sb = tsp.tile([P, 4 * P], fp16, tag=tg, name=f"T_{tg}_{i}")
                ps = tp_ps.tile([P, 4 * P], fp16, tag="tp",
                                name=f"tps_{tg}_{i}")
                for c in range(4):
                    nc.tensor.transpose(ps[:, c * P:(c + 1) * P],
                                        src[:, c * P:(c + 1) * P], ident)
                nc.scalar.copy(sb, ps)
                return sb

            hlT = tp16(hl_b, "ThL")
            hrT = tp16(hr_b, "ThR")
            hsT = tsp.tile([P, 4 * P], fp16, tag="ThS", name=f"T_ThS_{i}")
            nc.vector.tensor_add(hsT, hlT, hrT)
            hdT = tsp.tile([P, 4 * P], fp16, tag="ThD", name=f"T_ThD_{i}")
            nc.vector.tensor_sub(hdT, hlT, hrT)
            xhT = tp16(xh_b, "TxH")
            # xw: f32 transpose, downcast in the eviction.  Half-width
            # f32 PSUM groups (1KB) share the fp16 groups' pool slot.
            xwT = tsp.tile([P, 8 * P], fp16, tag="TxW", name=f"T_TxW_{i}")
            for g in range(4):
                ps = tp_ps.tile([P, 2 * P], f32, tag="tp",
                                name=f"tpw_{g}_{i}")
                for c in range(2):
                    nc.tensor.transpose(
                        ps[:, c * P:(c + 1) * P],
                        xw_t[:, (2 * g + c) * P:(2 * g + c + 1) * P], ident32)
                nc.scalar.copy(xwT[:, g * 2 * P:(g + 1) * 2 * P], ps)
            stT[i] = (hsT, hdT, xhT, xwT)

        def unit(tag, i):
            return mm_ps.tile([P, HALF], f32, tag="mm", name=f"ps_{tag}_{i}")

        def stage1m(i):
            """A+D matmuls and A-phase elementwise."""
            (hl_t, hr_t, xh_t, xw_t, hl_b, hr_b, xh_b, sS) = st0[i]
            (hsT, hdT, xhT, xwT) = stT[i]

            qS = tinyp.tile([P, 8], f32, tag="qS", name=f"qS_{i}")
            # ql/qr on scalar (Square + accum), cr2 on vector
            sg1 = scr.tile([P, HALF], fp16, tag="scr", name=f"scr_ql_{i}")
            nc.scalar.activation(sg1, hl_t, ACTF.Square,
                                 accum_out=qS[:, 0:1])
            sg2 = scr.tile([P, HALF], fp16, tag="scr", name=f"scr_qr_{i}")
            nc.scalar.activation(sg2, hr_t, ACTF.Square,
                                 accum_out=qS[:, 1:2])
            sg3 = scr.tile([P, HALF], fp16, tag="scr", name=f"scr_cr_{i}")
            nc.vector.scalar_tensor_tensor(sg3, hl_t, 0.0, hr_t, ALU.bypass,
                                           ALU.mult, accum_out=qS[:, 2:3])

            SUq, SBq, TU = unit("SUq", i), unit("SBq", i), unit("TU", i)
            for c in range(4):
                lhs = xhT[:, bass.ts(c, P)]
                st, sp_ = (c == 0), (c == 3)
                nc.tensor.matmul(SUq, lhs, wsb["qWu3"][:, c, :], start=st, stop=sp_)
                nc.tensor.matmul(SBq, lhs, wsb["qWbF"][:, c, :], start=st, stop=sp_)
                nc.tensor.matmul(TU, lhs, wsb["kWu3"][:, c, :], start=st, stop=sp_)
            CD = unit("CD", i)
            for c in range(4):
                nc.tensor.matmul(CD, hdT[:, bass.ts(c, P)],
                                 wsb["kU3"][:, c, :], start=(c == 0),
                                 stop=(c == 3))
            AS = unit("AS", i)
            for c in range(4):
                nc.tensor.matmul(AS, hsT[:, bass.ts(c, P)],
                                 wsb["qU3"][:, c, :], start=(c == 0),
                                 stop=(c == 3))
            AD = unit("AD", i)
            for c in range(4):
                nc.tensor.matmul(AD, hdT[:, bass.ts(c, P)],
                                 wsb["qU3"][:, c, :], start=(c == 0),
                                 stop=(c == 3))

            su = att.tile([P, HALF], fp16, tag="su", name=f"su_{i}")
            nc.vector.tensor_add(su, SUq, bc["b_qWu"])
            sbq = att.tile([P, HALF], fp16, tag="sbq", name=f"sbq_{i}")
            nc.vector.tensor_add(sbq, SBq, bc["b_qb"])
            tu = att.tile([P, HALF], fp16, tag="tu", name=f"tu_{i}")
            nc.vector.tensor_add(tu, TU, bc["b_kWu"])
            dk = att.tile([P, HALF], fp16, tag="dk", name=f"dk_{i}")
            nc.vector.tensor_mul(dk, CD, tu)
            u = att.tile([P, HALF], fp16, tag="u", name=f"u_{i}")
            nc.gpsimd.tensor_mul(u, su, dk)

            for j, (aa, bb) in enumerate([(sbq, dk), (AS, u), (AD, u)]):
                sdot = scr.tile([P, HALF], fp16, tag="scr_b",
                                name=f"scr_dot{j}_{i}")
                nc.vector.scalar_tensor_tensor(
                    sdot, aa, 0.0, bb, ALU.bypass, ALU.mult,
                    accum_out=qS[:, 3 + j:4 + j])

            # D-phase (attention-independent). Unit order tuned for the
            # 6-slot PSUM rotation: each alloc reuses a bank whose
            # consumer runs early.
            HSU, LSU = unit("HSU", i), unit("LSU", i)
            SBC = unit("SBC", i)
            for c in range(4):
                lhs = xhT[:, bass.ts(c, P)]
                st, sp_ = (c == 0), (c == 3)
                nc.tensor.matmul(HSU, lhs, wsb["hWu3"][:, c, :], start=st, stop=sp_)
                nc.tensor.matmul(LSU, lhs, wsb["lWu3"][:, c, :], start=st, stop=sp_)
                nc.tensor.matmul(SBC, lhs, wsb["WC3"][:, c, :], start=st, stop=sp_)
            LUp = unit("LU", i)
            for c in range(8):
                nc.tensor.matmul(LUp, xwT[:, bass.ts(c, P)],
                                 wsb["lU3"][:, c, :], start=(c == 0),
                                 stop=(c == 7))
            Mb = unit("Mb", i)
            for c in range(4):
                nc.tensor.matmul(Mb, hsT[:, bass.ts(c, P)],
                                 wsb["WS"][:, c, :], start=(c == 0),
                                 stop=(c == 3))
            D1 = unit("D1", i)
            for c in range(4):
                nc.tensor.matmul(D1, hdT[:, bass.ts(c, P)],
                                 wsb["WT"][:, c, :], start=(c == 0),
                                 stop=(c == 3))
            D2 = unit("D2", i)
            for c in range(4):
                nc.tensor.matmul(D2, hdT[:, bass.ts(c, P)],
                                 wsb["WB"][:, c, :], start=(c == 0),
                                 stop=(c == 3))
            return (qS, sS, HSU, LSU, SBC, LUp, Mb, D1, D2)

        def stage1b(i, h):
            (qS, sS, HSU, LSU, SBC, LUp, Mb, D1, D2) = h
            ql, qr, cr2 = qS[:, 0:1], qS[:, 1:2], qS[:, 2:3]
            cdt, sA, dA = qS[:, 3:4], qS[:, 4:5], qS[:, 5:6]
            sl, sr = sS[:, 0:1], sS[:, 1:2]

            # vector head: drain D-phase PSUM banks fast
            su_h = phd.tile([P, HALF], fp16, tag="su_h", name=f"su_h_{i}")
            nc.vector.tensor_add(su_h, HSU, bc["b_hWu"])
            su_l = phd.tile([P, HALF], fp16, tag="su_l", name=f"su_l_{i}")
            nc.vector.tensor_add(su_l, LSU, bc["b_lWu"])
            sbc = phd.tile([P, HALF], fp16, tag="sbc", name=f"sbc_{i}")
            nc.vector.tensor_add(sbc, SBC, bc["b_cb"])
            w1 = phd.tile([P, HALF], fp16, tag="w1", name=f"w1_{i}")
            nc.vector.tensor_mul(w1, LUp, su_l)
            # Mb eviction on scalar (frees its bank early; also needed
            # because DVE reads at most one PSUM operand per op)
            mb_sb = phd.tile([P, HALF], f32, tag="mb_sb", name=f"mb_{i}")
            nc.scalar.copy(mb_sb, Mb)

            # d0/d1 + two-term softsign sigmoid -> ab = [p00, -p11]
            ee = tinyp.tile([P, 2], f32, tag="ee", name=f"ee_{i}")
            nc.vector.tensor_add(ee[:, 0:1], sA, dA)
            nc.vector.scalar_tensor_tensor(ee[:, 1:2], dA, -1.0, sA,
                                           ALU.mult, ALU.add)
            dd = tinyp.tile([P, 2], f32, tag="dd", name=f"dd_{i}")
            nc.vector.scalar_tensor_tensor(dd, ee, 0.5,
                                           cdt.broadcast_to([P, 2]),
                                           ALU.mult, ALU.add)
            z2 = tinyp.tile([P, 2], f32, tag="z2", name=f"z2_{i}")
            nc.scalar.activation(z2, dd, ACTF.Square, scale=IS)
            sq1 = tinyp.tile([P, 2], f32, tag="sq1", name=f"sq1_{i}")
            nc.scalar.activation(sq1, z2, ACTF.Sqrt, scale=1.0 / SIG_C1,
                                 bias=1.0)
            sq2 = tinyp.tile([P, 2], f32, tag="sq2", name=f"sq2_{i}")
            nc.scalar.activation(sq2, z2, ACTF.Sqrt, scale=1.0 / SIG_C2,
                                 bias=1.0)
            rr = tinyp.tile([P, 4], f32, tag="rr", name=f"rr_{i}")
            nc.vector.reciprocal(rr[:, 0:2], sq1)
            nc.vector.reciprocal(rr[:, 2:4], sq2)
            mm_ = tinyp.tile([P, 2], f32, tag="mm2", name=f"mm2_{i}")
            nc.vector.scalar_tensor_tensor(mm_, rr[:, 0:2], SIG_K1 / SIG_K2,
                                           rr[:, 2:4], ALU.mult, ALU.add)
            psh = tinyp.tile([P, 2], f32, tag="psh", name=f"psh_{i}")
            nc.vector.scalar_tensor_tensor(psh, dd, SIG_K2, mm_,
                                           ALU.mult, ALU.mult)
            ab = tinyp.tile([P, 2], f32, tag="ab", name=f"ab_{i}")
            nc.vector.tensor_add(ab, psh, cb2)
            al, be = ab[:, 0:1], ab[:, 1:2]

            # D1/D2 drained by scalar as soon as al/be exist
            th0 = phd.tile([P, HALF], fp16, tag="th0", name=f"th0_{i}")
            nc.scalar.activation(th0, D1, ACTF.Copy, scale=al)
            th1 = phd.tile([P, HALF], fp16, tag="th1", name=f"th1_{i}")
            nc.scalar.activation(th1, D2, ACTF.Copy, scale=be)
            hv = phd.tile([P, HALF], fp16, tag="hv", name=f"hv_{i}")
            nc.gpsimd.tensor_add(hv, th0, th1)

            # layernorm stats
            gt = tinyp.tile([P, 8], f32, tag="gt", name=f"gt_{i}")
            g_, gh, dl, base = gt[:, 0:1], gt[:, 1:2], gt[:, 2:3], gt[:, 3:4]
            cA, cB, cC = gt[:, 4:5], gt[:, 5:6], gt[:, 6:7]
            nc.vector.tensor_add(g_, al, be)
            nc.vector.tensor_scalar(gh, g_, 0.5, None, ALU.mult)
            sqab = tinyp.tile([P, 2], f32, tag="sqab", name=f"sqab_{i}")
            nc.vector.tensor_mul(sqab, ab, ab)
            nc.vector.tensor_add(dl, sqab[:, 0:1], sqab[:, 1:2])
            nc.vector.tensor_scalar(base, dl, 0.5, 1.0, ALU.mult, ALU.add)
            nc.vector.tensor_add(cA, base, g_)
            nc.vector.scalar_tensor_tensor(cB, g_, -1.0, base, ALU.mult,
                                           ALU.add)
            nc.vector.tensor_scalar(cC, dl, -1.0, 2.0, ALU.mult, ALU.add)
            acc = tinyp.tile([P, 8], f32, tag="acc", name=f"acc_{i}")
            z0, z1, ssqh = acc[:, 0:1], acc[:, 1:2], acc[:, 2:3]
            sh2, sd2, sumxh = acc[:, 3:4], acc[:, 4:5], acc[:, 5:6]
            m2, varh = acc[:, 6:7], acc[:, 7:8]
            nc.vector.tensor_scalar_mul(z0, cr2, cC)
            nc.vector.scalar_tensor_tensor(z1, ql, cA, z0, ALU.mult, ALU.add)
            nc.vector.scalar_tensor_tensor(ssqh, qr, cB, z1, ALU.mult,
                                           ALU.add)
            nc.vector.tensor_add(sh2, sl, sr)
            nc.vector.tensor_sub(sd2, sl, sr)
            nc.vector.scalar_tensor_tensor(sumxh, sd2, gh, sh2, ALU.mult,
                                           ALU.add)
            nc.vector.tensor_mul(m2, sumxh, sumxh)
            nc.vector.scalar_tensor_tensor(varh, m2, -1.0 / 512.0, ssqh,
                                           ALU.mult, ALU.add)
            so = tinyp.tile([P, 2], f32, tag="so", name=f"so_{i}")
            sqstd, rinv = so[:, 0:1], so[:, 1:2]
            nc.scalar.activation(sqstd, varh, ACTF.Sqrt,
                                 scale=2.0 / (DIM - 1))
            nc.vector.reciprocal(rinv, sqstd)

            # t5 = (Mb + hv) - cs*mean ; u1 = rinv * t5
            t5a = phd.tile([P, HALF], f32, tag="t5a", name=f"t5a_{i}")
            nc.vector.scalar_tensor_tensor(t5a, bc["b_cs512"], sumxh, mb_sb,
                                           ALU.mult, ALU.subtract)
            t5 = phd.tile([P, HALF], fp16, tag="t5", name=f"t5_{i}")
            nc.vector.tensor_sub(t5, hv, t5a)
            u1 = phd.tile([P, HALF], fp16, tag="u1", name=f"u1_{i}")
            nc.scalar.activation(u1, t5, ACTF.Copy, scale=rinv)

            v1 = phd.tile([P, HALF], fp16, tag="v1", name=f"v1_{i}")
            nc.gpsimd.tensor_mul(v1, u1, su_h)
            s2 = phd.tile([P, HALF], fp16, tag="s2", name=f"s2_{i}")
            nc.gpsimd.tensor_add(s2, v1, sbc)
            out_t = outp.tile([P, HALF], f32, tag="out_t", name=f"out_{i}")
            nc.gpsimd.tensor_add(out_t, s2, w1)
            nc.sync.dma_start(out_d[bass.ts(i, P), :], out_t)

        stage0(0)
        stageT(0)
        stage0(1)
        stageT(1)
        for i in range(n_tiles):
            h = stage1m(i)
            if i + 2 < n_tiles:
                stage0(i + 2)
                stageT(i + 2)
            stage1b(i, h)

    nc.compile()
    return nc


_NC_CACHE = {}


def _get_nc(b_loc, mm_dt=None):
    if b_loc not in _NC_CACHE:
        _NC_CACHE[b_loc] = build_nc(b_loc)
    return _NC_CACHE[b_loc]


def make_in_maps(inputs):
    b = inputs["hl"].shape[0]
    b_loc = b // N_CORES
    prep = host_prep(inputs)
    in_maps = []
    for i in range(N_CORES):
        m = {}
        for k in ("hl", "hr", "xw", "xh"):
            v = np.ascontiguousarray(np.asarray(inputs[k], dtype=np.float32))
            m[k] = v[i * b_loc:(i + 1) * b_loc]
        m.update(prep)
        in_maps.append(m)
    return in_maps


def kernel(**inputs):
    b = inputs["hl"].shape[0]
    nc = _get_nc(b // N_CORES)
    in_maps = make_in_maps(inputs)
    res = run_bass_kernel_spmd(nc, in_maps, core_ids=list(range(N_CORES)))
    return np.concatenate([r["out"] for r in res.results], axis=0)
